# revision 20
# baseline (speedup 1.0000x reference)
"""Trainium2 Bass kernel for Llama GQA attention (no mask), 8-way tensor
parallel over KV heads.

Problem shapes (hardcoded):
  x  (2, 2048, 4096) f32
  wq (4096, 4096), wk (1024, 4096), wv (1024, 4096), wo (4096, 4096) f32
  NUM_HEADS=32, NUM_KV_HEADS=8, HEAD_DIM=128, GQA group g=4

Sharding: core c owns KV head c (4 Q heads). x replicated (pre-transposed
to xT on host), wq/wk/wv sharded on output dim (pre-transposed host-side),
wo sharded on input dim. Each core computes a partial (4096, 4096) output
(its heads' contribution through wo); host sums the 8 partials in fp32.

All tensors are bf16 (PSUM accumulation fp32): same PE rate as fp32r
(1 col/cycle) but half the DMA/SBUF traffic, which removes the phase-1
x-feed stalls the fp32 version had.

Structure:
  phase 1: q/k/v projections. Weight chunk k (wq+wk+wv) DMA'd as
    per-k tiles (dependency tracking is tile-granular) alternating
    gpsimd/scalar queues; x tiles on the sync queue (12-deep ring).
    wo prefetch is WAR-gated on a j==3 x tile so the run-ahead DMA
    queues can't flood the startup window. vT -> v via PE transposes.
    PSUM evacuations split ACT/DVE, k first (phase-2 boundary).
  phase 2 (fused attention + output projection, software-pipelined):
    per (batch, tq-chunk, head): scores transposed ST = kT_tile.T @ qT
    into [128,1024] PSUM (2 k-tiles per matmul pair), batched exp ->
    p (bf16). Softmax denominator = DVE pairwise-tree sum of the 16 p
    tiles + ONE ones-matmul per head (instead of 16 PE den matmuls).
    The PE is in-order, so per g the emission is: S-pair(g), then
    exp-independent filler (den flush of the previous head at g==1 /
    one output-projection H-chunk of the previous (b,chunk)), then
    PV-pair(g-1) — the filler covers the ACT exp latency. The last
    (b,chunk)'s output projection drains in a tail block.
"""

import sys
from contextlib import ExitStack

import numpy as np
from ml_dtypes import bfloat16

sys.path.insert(0, "/opt/trn_rl_repo")

import concourse.bass as bass  # noqa: E402
import concourse.tile as tile  # noqa: E402
from concourse import bacc, mybir  # noqa: E402
from concourse.bass_utils import run_bass_kernel_spmd  # noqa: E402
from concourse.masks import make_identity  # noqa: E402

NCORES = 8
B, S, H = 2, 2048, 4096
T = B * S                      # 4096 flattened tokens
D = 128                        # head dim
G = 4                          # q heads per core (GQA group)
HK = 32                        # h k-tiles (4096 / 128)
TT = T // 128                  # 32 token tiles
NJ = T // 512                  # 8 token chunks of 512
SJ = S // 512                  # 4 tq chunks per batch
SI = S // 128                  # 16 tk tiles per batch
NG = SI // 2                   # 8 k-tile pairs per batch
SCALE = float(1.0 / np.sqrt(D))

F32 = mybir.dt.float32
BF16 = mybir.dt.bfloat16
COPY = mybir.ActivationFunctionType.Copy
EXP = mybir.ActivationFunctionType.Exp


def build_nc():
    nc = bacc.Bacc("TRN2", target_bir_lowering=False, debug=False,
                   enable_asserts=True, num_devices=NCORES)
    xt = nc.declare_dram_parameter("xt", [H, T], BF16, isOutput=False)
    wqt = nc.declare_dram_parameter("wqt", [H, G * D], BF16, isOutput=False)
    wkt = nc.declare_dram_parameter("wkt", [H, D], BF16, isOutput=False)
    wvt = nc.declare_dram_parameter("wvt", [H, D], BF16, isOutput=False)
    wot = nc.declare_dram_parameter("wot", [G * D, H], BF16, isOutput=False)
    ones = nc.declare_dram_parameter("ones", [128, 128], BF16, isOutput=False)
    out = nc.declare_dram_parameter("out", [T, H], BF16, isOutput=True)

    xt_r = xt.ap().rearrange("(k p) t -> p k t", p=128)     # [128, 32, T]
    wqt_r = wqt.ap().rearrange("(k p) m -> p k m", p=128)   # [128, 32, 512]
    wkt_r = wkt.ap().rearrange("(k p) m -> p k m", p=128)   # [128, 32, 128]
    wvt_r = wvt.ap().rearrange("(k p) m -> p k m", p=128)   # [128, 32, 128]
    wot_r = wot.ap().rearrange("(k p) n -> p k n", p=128)   # [128, 4, T]
    out_r = out.ap()

    with tile.TileContext(nc) as tc:
        with ExitStack() as ctx:
            persist = ctx.enter_context(tc.tile_pool(name="persist", bufs=1))
            q_sb = persist.tile([128, G, T], BF16)       # qT per head, 4MB
            k_sb = persist.tile([128, T], BF16)          # kT, 1MB
            v_sb = persist.tile([128, TT, D], BF16)      # v natural, 1MB
            wo_sb = persist.tile([128, G, T], BF16)      # woT resident, 4MB
            ones_sb = persist.tile([128, 128], BF16)
            nc.scalar.dma_start(out=ones_sb, in_=ones.ap())

            # ---------------- phase 1: projections ----------------
            with ExitStack() as c1:
                wpool = c1.enter_context(tc.tile_pool(name="wpool", bufs=1))
                xpool = c1.enter_context(tc.tile_pool(name="xpool", bufs=12))
                vstg = c1.enter_context(tc.tile_pool(name="vstg", bufs=2))
                ps1 = c1.enter_context(tc.tile_pool(name="ps1", bufs=1, space="PSUM"))
                pstr = c1.enter_context(tc.tile_pool(name="pstr", bufs=2, space="PSUM"))

                # one tile PER k-chunk: dependency tracking is
                # tile-granular, so a single big tile would make the first
                # matmul wait for ALL 32 chunk DMAs
                wq_t = [wpool.tile([128, G * D], BF16, name=f"wq{k}")
                        for k in range(HK)]
                # wk/wv as quads of 4 k-chunks: 1KB DMA lines instead of
                # 256B, for better DMA efficiency in the bandwidth-bound
                # startup window
                wk_t = [wpool.tile([128, 4, D], BF16, name=f"wk{qi}")
                        for qi in range(HK // 4)]
                wv_t = [wpool.tile([128, 4, D], BF16, name=f"wv{qi}")
                        for qi in range(HK // 4)]
                ident = wpool.tile([128, 128], BF16)
                # alternate posts between the two free DMA queues in k
                # (= consumption) order so arrival tracks need
                posts = []
                for k in range(HK):
                    posts.append((wq_t[k], wqt_r[:, k, :]))
                    if k % 4 == 1:
                        qi = k // 4
                        posts.append((wk_t[qi], wkt_r[:, 4 * qi:4 * qi + 4, :]))
                        posts.append((wv_t[qi], wvt_r[:, 4 * qi:4 * qi + 4, :]))
                for i, (dst, src) in enumerate(posts):
                    q = nc.gpsimd if i % 2 == 0 else nc.scalar
                    q.dma_start(out=dst, in_=src)
                make_identity(nc, ident)

                def v_transpose(pj, pv_st):
                    # one-j-delayed so PE never waits on the DVE staging copy
                    vt_ps = pstr.tile([128, 4, 128], BF16)
                    for tt in range(4):
                        nc.tensor.transpose(
                            vt_ps[:, tt, :], pv_st[:, tt * 128:(tt + 1) * 128],
                            ident)
                    nc.scalar.activation(
                        out=v_sb[:, 4 * pj:4 * pj + 4, :], in_=vt_ps, func=COPY)

                prev_v = None
                for j in range(NJ):
                    tsl = slice(j * 512, (j + 1) * 512)
                    q_ps = [ps1.tile([128, 512], F32, name=f"q_ps{m}")
                            for m in range(G)]
                    k_ps = ps1.tile([128, 512], F32)
                    v_ps = ps1.tile([128, 512], F32)
                    for k in range(HK):
                        x_t = xpool.tile([128, 512], BF16)
                        nc.sync.dma_start(out=x_t, in_=xt_r[:, k, tsl])
                        st = k == 0
                        sp = k == HK - 1
                        for m in range(G):
                            nc.tensor.matmul(
                                q_ps[m], wq_t[k][:, m * D:(m + 1) * D], x_t,
                                start=st, stop=sp)
                        nc.tensor.matmul(k_ps, wk_t[k // 4][:, k % 4, :], x_t,
                                         start=st, stop=sp)
                        nc.tensor.matmul(v_ps, wv_t[k // 4][:, k % 4, :], x_t,
                                         start=st, stop=sp)
                        if k == 2 and prev_v is not None:
                            v_transpose(*prev_v)
                        # prefetch wo for phase 2, gated on a j==3 x tile.
                        # The gpsimd queue posts DMAs in relaxed order, so a
                        # copy BEFORE the dma_start does not delay it; a
                        # writer-after-writer dependency on wo_sb itself does.
                        if j == 3 and k == 0:
                            nc.vector.tensor_copy(wo_sb[0:1, 0, 0:1],
                                                  x_t[0:1, 0:1])
                            for kk in range(G):
                                nc.gpsimd.dma_start(out=wo_sb[:, kk, :],
                                                    in_=wot_r[:, kk, :])
                    # split psum evacuation across ACT and DVE so the banks
                    # free up fast for the next j iteration; v first so the
                    # delayed transpose never waits on the staging copy
                    v_st = vstg.tile([128, 512], BF16)
                    nc.vector.tensor_copy(v_st, v_ps)
                    nc.scalar.activation(out=k_sb[:, tsl], in_=k_ps, func=COPY)
                    nc.scalar.activation(out=q_sb[:, 0, tsl], in_=q_ps[0], func=COPY)
                    nc.vector.tensor_copy(q_sb[:, 1, tsl], q_ps[1])
                    nc.scalar.activation(out=q_sb[:, 2, tsl], in_=q_ps[2], func=COPY)
                    nc.vector.tensor_copy(q_sb[:, 3, tsl], q_ps[3])
                    prev_v = (j, v_st)
                v_transpose(*prev_v)

            # ------- phase 2: fused attention + output projection -------
            with ExitStack() as c2:
                apool = c2.enter_context(tc.tile_pool(name="apool", bufs=2))
                ppool = c2.enter_context(tc.tile_pool(name="ppool", bufs=4))
                tpool = c2.enter_context(tc.tile_pool(name="tpool", bufs=2))
                rpool = c2.enter_context(tc.tile_pool(name="rpool", bufs=2))
                opool = c2.enter_context(tc.tile_pool(name="opool", bufs=3))
                psS = c2.enter_context(tc.tile_pool(name="psS", bufs=2, space="PSUM"))
                psPV = c2.enter_context(tc.tile_pool(name="psPV", bufs=2, space="PSUM"))
                psO = c2.enter_context(tc.tile_pool(name="psO", bufs=2, space="PSUM"))

                evac_ctr = [0]

                def evac_o(o_ps, t0, n):
                    # alternate evacs over DVE/ACT (gpsimd cannot read PSUM
                    # on TRN2); a [128,512] evac takes ~690ns vs the 850ns
                    # 4-matmul group, so each engine sees one evac per 1.7us
                    o_t = opool.tile([128, 512], BF16)
                    e = evac_ctr[0] % 2
                    evac_ctr[0] += 1
                    if e == 0:
                        nc.vector.tensor_copy(o_t, o_ps)
                    else:
                        nc.scalar.activation(out=o_t, in_=o_ps, func=COPY)
                    nc.sync.dma_start(
                        out=out_r[t0:t0 + 128, n * 512:(n + 1) * 512],
                        in_=o_t)

                def outproj_nchunk(pb, pj, pa, tt2, n, o_ps=None):
                    # output projection for tq-tile tt2, H-chunk n, of chunk
                    # (pb, pj): accumulate the 4 heads in PSUM against woT.
                    t0 = pb * S + pj * 512 + tt2 * 128
                    if o_ps is None:
                        o_ps = psO.tile([128, 512], F32, name="o_ps")
                    for m in range(G):
                        nc.tensor.matmul(
                            o_ps, pa[m][:, tt2 * 128:(tt2 + 1) * 128],
                            wo_sb[:, m, n * 512:(n + 1) * 512],
                            start=(m == 0), stop=(m == G - 1))
                    evac_o(o_ps, t0, n)

                def flush_den(pend):
                    # softmax denominator of a finished head: one ones-matmul
                    # on the DVE tree sum, reciprocal, normalize into a_ch.
                    # The PSUM tile comes from the psO ring (outproj evacs
                    # release it fast) — a psS-ring tile would WAR-wait on
                    # the current head's exp(0), stalling the in-order PE.
                    acc512, pv_ps, a_t = pend
                    den_ps = psO.tile([128, 512], F32, name="o_ps")
                    nc.tensor.matmul(den_ps, ones_sb, acc512,
                                     start=True, stop=True)
                    rec_t = rpool.tile([128, 512], F32)
                    nc.vector.reciprocal_approx_fast(out=rec_t, in_=den_ps)
                    nc.vector.tensor_mul(a_t, pv_ps, rec_t)

                pending = None   # den work of the previous head
                prev = None      # a_ch of the previous (b, j)
                for b in range(B):
                    for j in range(SJ):
                        tqsl = slice(b * S + j * 512, b * S + (j + 1) * 512)
                        a_ch = [apool.tile([128, 512], BF16, name=f"a_ch{m}")
                                for m in range(G)]
                        for m in range(G):
                            pv_ps = psPV.tile([128, 512], F32,
                                              name="pv_ps")
                            p_list = []
                            t_parts = []
                            for g in range(NG):
                                s_ps = psS.tile([128, 1024], F32, name="s_ps")
                                for h in range(2):
                                    ti = b * SI + 2 * g + h
                                    nc.tensor.matmul(
                                        s_ps[:, h * 512:(h + 1) * 512],
                                        k_sb[:, ti * 128:(ti + 1) * 128],
                                        q_sb[:, m, tqsl], start=True, stop=True)
                                p_t = ppool.tile([128, 1024], BF16)
                                nc.scalar.activation(out=p_t, in_=s_ps,
                                                     func=EXP, scale=SCALE)
                                p_list.append(p_t)
                                if g % 2 == 1:
                                    tk = tpool.tile([128, 1024], BF16,
                                                    name=f"t{g // 2}")
                                    nc.vector.tensor_add(tk, p_list[g - 1],
                                                         p_list[g])
                                    t_parts.append(tk)
                                # den flush + interleaved output projection
                                # go BEFORE the PV pair: the PE is in-order,
                                # so exp-independent work must sit ahead of
                                # the exp-dependent PV matmuls to cover the
                                # ACT latency
                                # filler schedule. m==0 keeps the baseline
                                # order (den flush at g==1) because its
                                # g==1 outproj would read the prev chunk's
                                # a_ch[3], which this very flush writes.
                                # m>=1 move the den flush to g==2 with an
                                # outproj ahead of it at g==1: the previous
                                # head's DVE tree then has ~2.5us of PE work
                                # ahead of the den matmul instead of ~0.9us
                                # (den-waits-on-tree was most of the phase-2
                                # gap time), and the outproj also covers the
                                # exp(0) latency before PV(0).
                                if m == 0:
                                    if g == 1 and pending is not None:
                                        flush_den(pending)
                                        pending = None
                                    if g >= 2 and prev is not None:
                                        outproj_nchunk(prev[0], prev[1],
                                                       prev[2], m, g - 2)
                                else:
                                    if g == 1 and prev is not None:
                                        outproj_nchunk(prev[0], prev[1],
                                                       prev[2], m, 0)
                                    if g == 2 and pending is not None:
                                        flush_den(pending)
                                        pending = None
                                    if g >= 2 and prev is not None:
                                        outproj_nchunk(prev[0], prev[1],
                                                       prev[2], m, g - 1)
                                if g >= 1:
                                    pg = p_list[g - 1]
                                    for h in range(2):
                                        ti = b * SI + 2 * (g - 1) + h
                                        nc.tensor.matmul(
                                            pv_ps, v_sb[:, ti, :],
                                            pg[:, h * 512:(h + 1) * 512],
                                            start=(g == 1 and h == 0),
                                            stop=False)
                            if prev is not None:
                                outproj_nchunk(prev[0], prev[1], prev[2],
                                               m, NJ - 2 if m == 0 else NJ - 1)
                            pg = p_list[NG - 1]
                            for h in range(2):
                                ti = b * SI + 2 * (NG - 1) + h
                                nc.tensor.matmul(
                                    pv_ps, v_sb[:, ti, :],
                                    pg[:, h * 512:(h + 1) * 512],
                                    start=False, stop=(h == 1))
                            # finish the denominator tree on DVE
                            s0 = tpool.tile([128, 1024], BF16, name="s0")
                            s1 = tpool.tile([128, 1024], BF16, name="s1")
                            nc.vector.tensor_add(s0, t_parts[0], t_parts[1])
                            nc.vector.tensor_add(s1, t_parts[2], t_parts[3])
                            a1024 = tpool.tile([128, 1024], BF16, name="a1024")
                            nc.vector.tensor_add(a1024, s0, s1)
                            acc512 = tpool.tile([128, 512], BF16, name="a512")
                            nc.vector.tensor_add(acc512, a1024[:, 0:512],
                                                 a1024[:, 512:1024])
                            pending = (acc512, pv_ps, a_ch[m])
                            if m == 0 and prev is not None:
                                outproj_nchunk(prev[0], prev[1], prev[2],
                                               m, NJ - 1)
                        prev = (b, j, a_ch)
                # ---- tail: last chunk's den flush + output projection ----
                # Open two O groups with heads 0-2 first (their a_ch are
                # ready) so the in-order PE isn't stalled behind the last
                # head's den chain (DVE tree -> ones-matmul -> recip -> mul).
                # The den matmul uses a psPV-ring tile (PV is finished).
                pb, pj, pa = prev
                acc512, pv_ps, a_t = pending

                def tail_slot(i):
                    # 4-deep psum rotation for the tail: psO's 2 slots plus
                    # the (now idle) psS pool's 2 slots, so group i+4 waits
                    # on evac(i) with 3 groups of slack instead of 1
                    if i % 4 < 2:
                        return psO.tile([128, 512], F32, name="o_ps")
                    return psS.tile([128, 1024], F32, name="s_ps")[:, 0:512]

                first2 = ((0, 0), (1, 0))
                opened = []
                for gi, (tt2, n) in enumerate(first2):
                    o_ps = tail_slot(gi)
                    for m in range(3):
                        nc.tensor.matmul(
                            o_ps, pa[m][:, tt2 * 128:(tt2 + 1) * 128],
                            wo_sb[:, m, n * 512:(n + 1) * 512],
                            start=(m == 0), stop=False)
                    opened.append(o_ps)
                den_ps = psPV.tile([128, 512], F32, name="pv_ps")
                nc.tensor.matmul(den_ps, ones_sb, acc512, start=True,
                                 stop=True)
                rec_t = rpool.tile([128, 512], F32)
                nc.vector.reciprocal_approx_fast(out=rec_t, in_=den_ps)
                nc.vector.tensor_mul(a_t, pv_ps, rec_t)
                for gi, (tt2, n) in enumerate(first2):
                    o_ps = opened[gi]
                    nc.tensor.matmul(
                        o_ps, pa[3][:, tt2 * 128:(tt2 + 1) * 128],
                        wo_sb[:, 3, n * 512:(n + 1) * 512],
                        start=False, stop=True)
                    evac_o(o_ps, pb * S + pj * 512 + tt2 * 128, n)
                ti = 2
                for tt2 in range(4):
                    for n in range(NJ):
                        if (tt2, n) in first2:
                            continue
                        outproj_nchunk(pb, pj, pa, tt2, n,
                                       o_ps=tail_slot(ti))
                        ti += 1
    nc.compile()
    return nc


_NC_CACHE = None


def _get_nc():
    global _NC_CACHE
    if _NC_CACHE is None:
        _NC_CACHE = build_nc()
    return _NC_CACHE


def make_in_maps(x, wq, wk, wv, wo):
    xt = np.ascontiguousarray(x.reshape(T, H).T).astype(bfloat16)
    wqb = wq.astype(bfloat16)
    wkb = wk.astype(bfloat16)
    wvb = wv.astype(bfloat16)
    wob = wo.astype(bfloat16)
    ones = np.ones((128, 128), dtype=bfloat16)
    in_maps = []
    for c in range(NCORES):
        qsl = slice(c * G * D, (c + 1) * G * D)
        ksl = slice(c * D, (c + 1) * D)
        in_maps.append({
            "xt": xt,
            "wqt": np.ascontiguousarray(wqb[qsl, :].T),
            "wkt": np.ascontiguousarray(wkb[ksl, :].T),
            "wvt": np.ascontiguousarray(wvb[ksl, :].T),
            "wot": np.ascontiguousarray(wob[:, qsl].T),
            "ones": ones,
        })
    return in_maps


def kernel(x, wq, wk, wv, wo, **run_kwargs):
    nc = _get_nc()
    in_maps = make_in_maps(np.asarray(x, dtype=np.float32),
                           np.asarray(wq, dtype=np.float32),
                           np.asarray(wk, dtype=np.float32),
                           np.asarray(wv, dtype=np.float32),
                           np.asarray(wo, dtype=np.float32))
    res = run_bass_kernel_spmd(nc, in_maps, core_ids=list(range(NCORES)),
                               **run_kwargs)
    acc = np.zeros((T, H), dtype=np.float32)
    for c in range(NCORES):
        acc += res.results[c]["out"].astype(np.float32)
    out = acc.reshape(B, S, H)
    if run_kwargs:
        return out, res
    return out



# revision 33
# speedup vs baseline: 1.1896x; 1.1896x over previous
"""Trainium2 Bass kernel for Llama GQA attention (no mask), 8-way tensor
parallel over KV heads.

Problem shapes (hardcoded):
  x  (2, 2048, 4096) f32
  wq (4096, 4096), wk (1024, 4096), wv (1024, 4096), wo (4096, 4096) f32
  NUM_HEADS=32, NUM_KV_HEADS=8, HEAD_DIM=128, GQA group g=4

Sharding: core c owns KV head c (4 Q heads). x replicated (pre-transposed
to xT on host), wq/wk/wv sharded on output dim (pre-transposed host-side),
wo sharded on input dim. Each core computes a partial (4096, 4096) output
(its heads' contribution through wo); host sums the 8 partials in fp32.

All tensors are bf16 (PSUM accumulation fp32): same PE rate as fp32r
(1 col/cycle) but half the DMA/SBUF traffic, which removes the phase-1
x-feed stalls the fp32 version had.

Structure:
  phase 1: q/k/v projections. Weight chunk k (wq+wk+wv) DMA'd as
    per-k tiles (dependency tracking is tile-granular) alternating
    gpsimd/scalar queues; x tiles on the sync queue (12-deep ring).
    wo prefetch is WAR-gated on a j==3 x tile so the run-ahead DMA
    queues can't flood the startup window. vT -> v via PE transposes.
    PSUM evacuations split ACT/DVE, k first (phase-2 boundary).
  phase 2 (fused attention + output projection, software-pipelined):
    per (batch, tq-chunk, head): scores transposed ST = kT_tile.T @ qT
    into [128,1024] PSUM (2 k-tiles per matmul pair), batched exp ->
    p (bf16). Softmax denominator = DVE pairwise-tree sum of the 16 p
    tiles + ONE ones-matmul per head (instead of 16 PE den matmuls).
    The PE is in-order, so per g the emission is: S-pair(g), then
    exp-independent filler (den flush of the previous head / one
    output-projection H-chunk of the previous (b,chunk)), then
    PV-pair(g-1) — the filler covers the ACT exp latency. For heads
    m>=1 the den flush sits at g==2 behind an outproj at g==1, giving
    the previous head's DVE tree ~2.5us of PE cover (den-waits-on-tree
    was most of the phase-2 gap time); m==0 keeps the flush at g==1
    because its g==1 outproj would read the a_ch[3] that this flush
    writes. Every head ends with an outproj after its tree so the next
    den matmul never leads the tree. The last (b,chunk)'s output
    projection drains in a tail block with a 4-deep PSUM rotation
    (psO's 2 slots + the idle psS pool's 2).

    Measured constraints (this session): fp8e4m3 DoubleRow = 2x flops
    only (216ns for K=256 x 512 cols, same as bf16 K=128) and any
    single-fp8 operand costs 3e-2..7e-2 rel err (budget 2e-2), so fp8
    cannot beat the 766us bf16 PE floor here. DMA aggregate ~244GB/s
    regardless of line size; the j=0 window (weights 6MB + x 4MB in
    41us) is at capacity, so ~12us of early PE stalls are structural.
    GPSIMD cannot read PSUM. Splitting PSUM accumulation groups with
    interleaved matmuls costs ~0.6us per resume - avoid open groups.
"""

import sys
from contextlib import ExitStack

import numpy as np
from ml_dtypes import bfloat16

sys.path.insert(0, "/opt/trn_rl_repo")

import concourse.bass as bass  # noqa: E402
import concourse.tile as tile  # noqa: E402
from concourse import bacc, mybir  # noqa: E402
from concourse.bass_utils import run_bass_kernel_spmd  # noqa: E402
from concourse.masks import make_identity  # noqa: E402

NCORES = 8
B, S, H = 2, 2048, 4096
T = B * S                      # 4096 flattened tokens
D = 128                        # head dim
G = 4                          # q heads per core (GQA group)
HK = 32                        # h k-tiles (4096 / 128)
TT = T // 128                  # 32 token tiles
NJ = T // 512                  # 8 token chunks of 512
SJ = S // 512                  # 4 tq chunks per batch
SI = S // 128                  # 16 tk tiles per batch
NG = SI // 2                   # 8 k-tile pairs per batch
SCALE = float(1.0 / np.sqrt(D))

F32 = mybir.dt.float32
BF16 = mybir.dt.bfloat16
COPY = mybir.ActivationFunctionType.Copy
EXP = mybir.ActivationFunctionType.Exp


def build_nc():
    nc = bacc.Bacc("TRN2", target_bir_lowering=False, debug=False,
                   enable_asserts=True, num_devices=NCORES)
    xt = nc.declare_dram_parameter("xt", [H, T], BF16, isOutput=False)
    wqt = nc.declare_dram_parameter("wqt", [H, G * D], BF16, isOutput=False)
    wkt = nc.declare_dram_parameter("wkt", [H, D], BF16, isOutput=False)
    wvt = nc.declare_dram_parameter("wvt", [H, D], BF16, isOutput=False)
    wot = nc.declare_dram_parameter("wot", [G * D, H], BF16, isOutput=False)
    ones = nc.declare_dram_parameter("ones", [128, 128], BF16, isOutput=False)
    out = nc.declare_dram_parameter("out", [T, H], BF16, isOutput=True)

    xt_r = xt.ap().rearrange("(k p) t -> p k t", p=128)     # [128, 32, T]
    wqt_r = wqt.ap().rearrange("(k p) m -> p k m", p=128)   # [128, 32, 512]
    wkt_r = wkt.ap().rearrange("(k p) m -> p k m", p=128)   # [128, 32, 128]
    wvt_r = wvt.ap().rearrange("(k p) m -> p k m", p=128)   # [128, 32, 128]
    wot_r = wot.ap().rearrange("(k p) n -> p k n", p=128)   # [128, 4, T]
    out_r = out.ap()

    with tile.TileContext(nc) as tc:
        with ExitStack() as ctx:
            persist = ctx.enter_context(tc.tile_pool(name="persist", bufs=1))
            q_sb = persist.tile([128, G, T], BF16)       # qT per head, 4MB
            k_sb = persist.tile([128, T], BF16)          # kT, 1MB
            v_sb = persist.tile([128, TT, D], BF16)      # v natural, 1MB
            wo_sb = persist.tile([128, G, T], BF16)      # woT resident, 4MB
            ones_sb = persist.tile([128, 128], BF16)

            # ---------------- phase 1: projections ----------------
            with ExitStack() as c1:
                wpool = c1.enter_context(tc.tile_pool(name="wpool", bufs=1))
                xpool = c1.enter_context(tc.tile_pool(name="xpool", bufs=12))
                vstg = c1.enter_context(tc.tile_pool(name="vstg", bufs=2))
                ps1 = c1.enter_context(tc.tile_pool(name="ps1", bufs=1, space="PSUM"))
                pstr = c1.enter_context(tc.tile_pool(name="pstr", bufs=2, space="PSUM"))

                # one tile PER k-chunk: dependency tracking is
                # tile-granular, so a single big tile would make the first
                # matmul wait for ALL 32 chunk DMAs
                wq_t = [wpool.tile([128, G * D], BF16, name=f"wq{k}")
                        for k in range(HK)]
                # wk/wv as quads of 4 k-chunks: 1KB DMA lines instead of
                # 256B, for better DMA efficiency in the bandwidth-bound
                # startup window
                wk_t = [wpool.tile([128, 4, D], BF16, name=f"wk{qi}")
                        for qi in range(HK // 4)]
                wv_t = [wpool.tile([128, 4, D], BF16, name=f"wv{qi}")
                        for qi in range(HK // 4)]
                ident = wpool.tile([128, 128], BF16)
                # alternate posts between the two free DMA queues in k
                # (= consumption) order so arrival tracks need
                posts = []
                for k in range(HK):
                    posts.append((wq_t[k], wqt_r[:, k, :]))
                    if k % 4 == 1:
                        qi = k // 4
                        posts.append((wk_t[qi], wkt_r[:, 4 * qi:4 * qi + 4, :]))
                        posts.append((wv_t[qi], wvt_r[:, 4 * qi:4 * qi + 4, :]))
                for i, (dst, src) in enumerate(posts):
                    q = nc.gpsimd if i % 2 == 0 else nc.scalar
                    q.dma_start(out=dst, in_=src)
                nc.scalar.dma_start(out=ones_sb, in_=ones.ap())
                make_identity(nc, ident)

                def v_transpose(pj, pv_st):
                    # one-j-delayed so PE never waits on the DVE staging copy
                    vt_ps = pstr.tile([128, 4, 128], BF16)
                    for tt in range(4):
                        nc.tensor.transpose(
                            vt_ps[:, tt, :], pv_st[:, tt * 128:(tt + 1) * 128],
                            ident)
                    nc.scalar.activation(
                        out=v_sb[:, 4 * pj:4 * pj + 4, :], in_=vt_ps, func=COPY)

                prev_v = None
                for j in range(NJ):
                    tsl = slice(j * 512, (j + 1) * 512)
                    q_ps = [ps1.tile([128, 512], F32, name=f"q_ps{m}")
                            for m in range(G)]
                    k_ps = ps1.tile([128, 512], F32)
                    v_ps = ps1.tile([128, 512], F32)
                    for k in range(HK):
                        x_t = xpool.tile([128, 512], BF16)
                        nc.sync.dma_start(out=x_t, in_=xt_r[:, k, tsl])
                        st = k == 0
                        sp = k == HK - 1
                        for m in range(G):
                            nc.tensor.matmul(
                                q_ps[m], wq_t[k][:, m * D:(m + 1) * D], x_t,
                                start=st, stop=sp)
                        nc.tensor.matmul(k_ps, wk_t[k // 4][:, k % 4, :], x_t,
                                         start=st, stop=sp)
                        nc.tensor.matmul(v_ps, wv_t[k // 4][:, k % 4, :], x_t,
                                         start=st, stop=sp)
                        if k == 2 and prev_v is not None:
                            v_transpose(*prev_v)
                        # prefetch wo for phase 2, gated on a j==3 x tile.
                        # The gpsimd queue posts DMAs in relaxed order, so a
                        # copy BEFORE the dma_start does not delay it; a
                        # writer-after-writer dependency on wo_sb itself does.
                        if j == 3 and k == 0:
                            nc.vector.tensor_copy(wo_sb[0:1, 0, 0:1],
                                                  x_t[0:1, 0:1])
                            for kk in range(G):
                                nc.gpsimd.dma_start(out=wo_sb[:, kk, :],
                                                    in_=wot_r[:, kk, :])
                    # split psum evacuation across ACT and DVE so the banks
                    # free up fast for the next j iteration; v first so the
                    # delayed transpose never waits on the staging copy.
                    # For the last j, evac q0/q1 first instead: phase 2's
                    # psS pool lands on the banks allocated first in this
                    # scope (q_ps0/q_ps1), so freeing those first lets the
                    # first S matmuls start ~1.4us earlier.
                    v_st = vstg.tile([128, 512], BF16)
                    if j < NJ - 1:
                        nc.vector.tensor_copy(v_st, v_ps)
                        nc.scalar.activation(out=k_sb[:, tsl], in_=k_ps,
                                             func=COPY)
                        nc.scalar.activation(out=q_sb[:, 0, tsl],
                                             in_=q_ps[0], func=COPY)
                        nc.vector.tensor_copy(q_sb[:, 1, tsl], q_ps[1])
                        nc.scalar.activation(out=q_sb[:, 2, tsl],
                                             in_=q_ps[2], func=COPY)
                        nc.vector.tensor_copy(q_sb[:, 3, tsl], q_ps[3])
                    else:
                        nc.scalar.activation(out=q_sb[:, 0, tsl],
                                             in_=q_ps[0], func=COPY)
                        nc.vector.tensor_copy(q_sb[:, 1, tsl], q_ps[1])
                        nc.vector.tensor_copy(v_st, v_ps)
                        nc.scalar.activation(out=k_sb[:, tsl], in_=k_ps,
                                             func=COPY)
                        nc.scalar.activation(out=q_sb[:, 2, tsl],
                                             in_=q_ps[2], func=COPY)
                        nc.vector.tensor_copy(q_sb[:, 3, tsl], q_ps[3])
                    prev_v = (j, v_st)
                v_transpose(*prev_v)

            # ------- phase 2: fused attention + output projection -------
            with ExitStack() as c2:
                apool = c2.enter_context(tc.tile_pool(name="apool", bufs=2))
                ppool = c2.enter_context(tc.tile_pool(name="ppool", bufs=4))
                tpool = c2.enter_context(tc.tile_pool(name="tpool", bufs=2))
                rpool = c2.enter_context(tc.tile_pool(name="rpool", bufs=2))
                opool = c2.enter_context(tc.tile_pool(name="opool", bufs=3))
                psS = c2.enter_context(tc.tile_pool(name="psS", bufs=2, space="PSUM"))
                psPV = c2.enter_context(tc.tile_pool(name="psPV", bufs=2, space="PSUM"))
                psO = c2.enter_context(tc.tile_pool(name="psO", bufs=2, space="PSUM"))

                evac_ctr = [0]

                def evac_o(o_ps, t0, n):
                    # alternate evacs over DVE/ACT by n-parity (gpsimd
                    # cannot read PSUM on TRN2). Even n (incl. the n=6
                    # filler right before a head's den tree finishes) goes
                    # to ACT so it cannot queue ahead of the tree's final
                    # adds on DVE; odd n (incl. n=7, emitted after the
                    # tree) goes to DVE.
                    o_t = opool.tile([128, 512], BF16)
                    if n % 2 == 0:
                        nc.scalar.activation(out=o_t, in_=o_ps, func=COPY)
                    else:
                        nc.vector.tensor_copy(o_t, o_ps)
                    nc.sync.dma_start(
                        out=out_r[t0:t0 + 128, n * 512:(n + 1) * 512],
                        in_=o_t)

                def outproj_nchunk(pb, pj, pa, tt2, n, o_ps=None):
                    # output projection for tq-tile tt2, H-chunk n, of chunk
                    # (pb, pj): accumulate the 4 heads in PSUM against woT.
                    t0 = pb * S + pj * 512 + tt2 * 128
                    if o_ps is None:
                        o_ps = psO.tile([128, 512], F32, name="o_ps")
                    for m in range(G):
                        nc.tensor.matmul(
                            o_ps, pa[m][:, tt2 * 128:(tt2 + 1) * 128],
                            wo_sb[:, m, n * 512:(n + 1) * 512],
                            start=(m == 0), stop=(m == G - 1))
                    evac_o(o_ps, t0, n)

                def flush_den(pend):
                    # softmax denominator of a finished head: one ones-matmul
                    # on the DVE tree sum, reciprocal, normalize into a_ch.
                    # The PSUM tile comes from the psO ring (outproj evacs
                    # release it fast) — a psS-ring tile would WAR-wait on
                    # the current head's exp(0), stalling the in-order PE.
                    acc512, pv_ps, a_t = pend
                    den_ps = psO.tile([128, 512], F32, name="o_ps")
                    nc.tensor.matmul(den_ps, ones_sb, acc512,
                                     start=True, stop=True)
                    rec_t = rpool.tile([128, 512], F32)
                    nc.vector.reciprocal_approx_fast(out=rec_t, in_=den_ps)
                    nc.vector.tensor_mul(a_t, pv_ps, rec_t)

                pending = None   # den work of the previous head
                prev = None      # a_ch of the previous (b, j)
                for b in range(B):
                    for j in range(SJ):
                        tqsl = slice(b * S + j * 512, b * S + (j + 1) * 512)
                        a_ch = [apool.tile([128, 512], BF16, name=f"a_ch{m}")
                                for m in range(G)]
                        for m in range(G):
                            pv_ps = psPV.tile([128, 512], F32,
                                              name="pv_ps")
                            p_list = []
                            t_parts = []
                            for g in range(NG):
                                s_ps = psS.tile([128, 1024], F32, name="s_ps")
                                for h in range(2):
                                    ti = b * SI + 2 * g + h
                                    nc.tensor.matmul(
                                        s_ps[:, h * 512:(h + 1) * 512],
                                        k_sb[:, ti * 128:(ti + 1) * 128],
                                        q_sb[:, m, tqsl], start=True, stop=True)
                                p_t = ppool.tile([128, 1024], BF16)
                                nc.scalar.activation(out=p_t, in_=s_ps,
                                                     func=EXP, scale=SCALE)
                                p_list.append(p_t)
                                if g % 2 == 1:
                                    tk = tpool.tile([128, 1024], BF16,
                                                    name=f"t{g // 2}")
                                    nc.vector.tensor_add(tk, p_list[g - 1],
                                                         p_list[g])
                                    t_parts.append(tk)
                                    # pre-reduce the den tree as tiles land
                                    # so only t3 -> a1024 -> acc512 remain
                                    # after exp(7): the den matmul of the
                                    # next head stalled ~0.6us on this chain
                                    if g == 3:
                                        pre = tpool.tile([128, 1024], BF16,
                                                         name="pre")
                                        nc.vector.tensor_add(
                                            pre, t_parts[0], t_parts[1])
                                    elif g == 5:
                                        pre2 = tpool.tile([128, 1024], BF16,
                                                          name="pre2")
                                        nc.vector.tensor_add(
                                            pre2, pre, t_parts[2])
                                # den flush + interleaved output projection
                                # go BEFORE the PV pair: the PE is in-order,
                                # so exp-independent work must sit ahead of
                                # the exp-dependent PV matmuls to cover the
                                # ACT latency
                                # filler schedule. m==0 keeps the baseline
                                # order (den flush at g==1) because its
                                # g==1 outproj would read the prev chunk's
                                # a_ch[3], which this very flush writes.
                                # m>=1 move the den flush to g==2 with an
                                # outproj ahead of it at g==1: the previous
                                # head's DVE tree then has ~2.5us of PE work
                                # ahead of the den matmul instead of ~0.9us
                                # (den-waits-on-tree was most of the phase-2
                                # gap time), and the outproj also covers the
                                # exp(0) latency before PV(0).
                                if m == 0:
                                    if g == 1 and pending is not None:
                                        flush_den(pending)
                                        pending = None
                                    if g >= 2 and prev is not None:
                                        outproj_nchunk(prev[0], prev[1],
                                                       prev[2], m, g - 2)
                                else:
                                    if g == 1 and prev is not None:
                                        outproj_nchunk(prev[0], prev[1],
                                                       prev[2], m, 0)
                                    if g == 2 and pending is not None:
                                        flush_den(pending)
                                        pending = None
                                    if g >= 2 and prev is not None:
                                        outproj_nchunk(prev[0], prev[1],
                                                       prev[2], m, g - 1)
                                if g >= 1:
                                    pg = p_list[g - 1]
                                    for h in range(2):
                                        ti = b * SI + 2 * (g - 1) + h
                                        nc.tensor.matmul(
                                            pv_ps, v_sb[:, ti, :],
                                            pg[:, h * 512:(h + 1) * 512],
                                            start=(g == 1 and h == 0),
                                            stop=False)
                            if m == 0 and prev is not None:
                                outproj_nchunk(prev[0], prev[1], prev[2],
                                               m, NJ - 2)
                            pg = p_list[NG - 1]
                            for h in range(2):
                                ti = b * SI + 2 * (NG - 1) + h
                                nc.tensor.matmul(
                                    pv_ps, v_sb[:, ti, :],
                                    pg[:, h * 512:(h + 1) * 512],
                                    start=False, stop=(h == 1))
                            # finish the denominator tree on DVE
                            a1024 = tpool.tile([128, 1024], BF16, name="a1024")
                            nc.vector.tensor_add(a1024, pre2, t_parts[3])
                            acc512 = tpool.tile([128, 512], BF16, name="a512")
                            nc.vector.tensor_add(acc512, a1024[:, 0:512],
                                                 a1024[:, 512:1024])
                            pending = (acc512, pv_ps, a_ch[m])
                            # post-pending filler for every head: covers the
                            # NEXT head's den matmul (m==0: tree of the prev
                            # chunk's last head) against the DVE tree latency
                            if prev is not None:
                                outproj_nchunk(prev[0], prev[1], prev[2],
                                               m, NJ - 1)
                        prev = (b, j, a_ch)
                # ---- tail: last chunk's den flush + output projection ----
                # Open two O groups with heads 0-2 first (their a_ch are
                # ready) so the in-order PE isn't stalled behind the last
                # head's den chain (DVE tree -> ones-matmul -> recip -> mul).
                # The den matmul uses a psPV-ring tile (PV is finished).
                pb, pj, pa = prev
                acc512, pv_ps, a_t = pending

                def tail_slot(i):
                    # 4-deep psum rotation for the tail: psO's 2 slots plus
                    # the (now idle) psS pool's 2 slots, so group i+4 waits
                    # on evac(i) with 3 groups of slack instead of 1
                    if i % 4 < 2:
                        return psO.tile([128, 512], F32, name="o_ps")
                    return psS.tile([128, 1024], F32, name="s_ps")[:, 0:512]

                den_ps = psPV.tile([128, 512], F32, name="pv_ps")
                nc.tensor.matmul(den_ps, ones_sb, acc512, start=True,
                                 stop=True)
                rec_t = rpool.tile([128, 512], F32)
                nc.vector.reciprocal_approx_fast(out=rec_t, in_=den_ps)
                nc.vector.tensor_mul(a_t, pv_ps, rec_t)
                ti = 0
                for tt2 in range(4):
                    for n in range(NJ):
                        outproj_nchunk(pb, pj, pa, tt2, n,
                                       o_ps=tail_slot(ti))
                        ti += 1
    nc.compile()
    return nc


_NC_CACHE = None


def _get_nc():
    global _NC_CACHE
    if _NC_CACHE is None:
        _NC_CACHE = build_nc()
    return _NC_CACHE


def make_in_maps(x, wq, wk, wv, wo):
    xt = np.ascontiguousarray(x.reshape(T, H).T).astype(bfloat16)
    wqb = wq.astype(bfloat16)
    wkb = wk.astype(bfloat16)
    wvb = wv.astype(bfloat16)
    wob = wo.astype(bfloat16)
    ones = np.ones((128, 128), dtype=bfloat16)
    in_maps = []
    for c in range(NCORES):
        qsl = slice(c * G * D, (c + 1) * G * D)
        ksl = slice(c * D, (c + 1) * D)
        in_maps.append({
            "xt": xt,
            "wqt": np.ascontiguousarray(wqb[qsl, :].T),
            "wkt": np.ascontiguousarray(wkb[ksl, :].T),
            "wvt": np.ascontiguousarray(wvb[ksl, :].T),
            "wot": np.ascontiguousarray(wob[:, qsl].T),
            "ones": ones,
        })
    return in_maps


def kernel(x, wq, wk, wv, wo, **run_kwargs):
    nc = _get_nc()
    in_maps = make_in_maps(np.asarray(x, dtype=np.float32),
                           np.asarray(wq, dtype=np.float32),
                           np.asarray(wk, dtype=np.float32),
                           np.asarray(wv, dtype=np.float32),
                           np.asarray(wo, dtype=np.float32))
    res = run_bass_kernel_spmd(nc, in_maps, core_ids=list(range(NCORES)),
                               **run_kwargs)
    acc = np.zeros((T, H), dtype=np.float32)
    for c in range(NCORES):
        acc += res.results[c]["out"].astype(np.float32)
    out = acc.reshape(B, S, H)
    if run_kwargs:
        return out, res
    return out



# revision 36
# speedup vs baseline: 1.1897x; 1.0001x over previous
"""Trainium2 Bass kernel for Llama GQA attention (no mask), 8-way tensor
parallel over KV heads.

Problem shapes (hardcoded):
  x  (2, 2048, 4096) f32
  wq (4096, 4096), wk (1024, 4096), wv (1024, 4096), wo (4096, 4096) f32
  NUM_HEADS=32, NUM_KV_HEADS=8, HEAD_DIM=128, GQA group g=4

Sharding: core c owns KV head c (4 Q heads). x replicated (pre-transposed
to xT on host), wq/wk/wv sharded on output dim (pre-transposed host-side),
wo sharded on input dim. Each core computes a partial (4096, 4096) output
(its heads' contribution through wo); host sums the 8 partials in fp32.

All tensors are bf16 (PSUM accumulation fp32): same PE rate as fp32r
(1 col/cycle) but half the DMA/SBUF traffic, which removes the phase-1
x-feed stalls the fp32 version had.

Structure:
  phase 1: q/k/v projections. Weight chunk k (wq+wk+wv) DMA'd as
    per-k tiles (dependency tracking is tile-granular) alternating
    gpsimd/scalar queues; x tiles on the sync queue (12-deep ring).
    wo prefetch is WAR-gated on a j==3 x tile so the run-ahead DMA
    queues can't flood the startup window. vT -> v via PE transposes.
    PSUM evacuations split ACT/DVE, k first (phase-2 boundary).
  phase 2 (fused attention + output projection, software-pipelined):
    per (batch, tq-chunk, head): scores transposed ST = kT_tile.T @ qT
    into [128,1024] PSUM (2 k-tiles per matmul pair), batched exp ->
    p (bf16). Softmax denominator = DVE pairwise-tree sum of the 16 p
    tiles + ONE ones-matmul per head (instead of 16 PE den matmuls).
    The PE is in-order, so per g the emission is: S-pair(g), then
    exp-independent filler (den flush of the previous head / one
    output-projection H-chunk of the previous (b,chunk)), then
    PV-pair(g-1) — the filler covers the ACT exp latency. For heads
    m>=1 the den flush sits at g==2 behind an outproj at g==1, giving
    the previous head's DVE tree ~2.5us of PE cover (den-waits-on-tree
    was most of the phase-2 gap time); m==0 keeps the flush at g==1
    because its g==1 outproj would read the a_ch[3] that this flush
    writes. Every head ends with an outproj after its tree so the next
    den matmul never leads the tree. The last (b,chunk)'s output
    projection drains in a tail block with a 4-deep PSUM rotation
    (psO's 2 slots + the idle psS pool's 2).

    Measured constraints (this session): fp8e4m3 DoubleRow = 2x flops
    only (216ns for K=256 x 512 cols, same as bf16 K=128) and any
    single-fp8 operand costs 3e-2..7e-2 rel err (budget 2e-2), so fp8
    cannot beat the 766us bf16 PE floor here. DMA aggregate ~244GB/s
    regardless of line size; the j=0 window (weights 6MB + x 4MB in
    41us) is at capacity, so ~12us of early PE stalls are structural.
    GPSIMD cannot read PSUM. Splitting PSUM accumulation groups with
    interleaved matmuls costs ~0.6us per resume - avoid open groups.
"""

import sys
from contextlib import ExitStack

import numpy as np
from ml_dtypes import bfloat16

sys.path.insert(0, "/opt/trn_rl_repo")

import concourse.bass as bass  # noqa: E402
import concourse.tile as tile  # noqa: E402
from concourse import bacc, mybir  # noqa: E402
from concourse.bass_utils import run_bass_kernel_spmd  # noqa: E402
from concourse.masks import make_identity  # noqa: E402

NCORES = 8
B, S, H = 2, 2048, 4096
T = B * S                      # 4096 flattened tokens
D = 128                        # head dim
G = 4                          # q heads per core (GQA group)
HK = 32                        # h k-tiles (4096 / 128)
TT = T // 128                  # 32 token tiles
NJ = T // 512                  # 8 token chunks of 512
SJ = S // 512                  # 4 tq chunks per batch
SI = S // 128                  # 16 tk tiles per batch
NG = SI // 2                   # 8 k-tile pairs per batch
SCALE = float(1.0 / np.sqrt(D))

F32 = mybir.dt.float32
BF16 = mybir.dt.bfloat16
COPY = mybir.ActivationFunctionType.Copy
EXP = mybir.ActivationFunctionType.Exp


def build_nc():
    nc = bacc.Bacc("TRN2", target_bir_lowering=False, debug=False,
                   enable_asserts=True, num_devices=NCORES)
    xt = nc.declare_dram_parameter("xt", [H, T], BF16, isOutput=False)
    wqt = nc.declare_dram_parameter("wqt", [H, G * D], BF16, isOutput=False)
    wkt = nc.declare_dram_parameter("wkt", [H, D], BF16, isOutput=False)
    wvt = nc.declare_dram_parameter("wvt", [H, D], BF16, isOutput=False)
    wot = nc.declare_dram_parameter("wot", [G * D, H], BF16, isOutput=False)
    ones = nc.declare_dram_parameter("ones", [128, 128], BF16, isOutput=False)
    out = nc.declare_dram_parameter("out", [T, H], BF16, isOutput=True)

    xt_r = xt.ap().rearrange("(k p) t -> p k t", p=128)     # [128, 32, T]
    wqt_r = wqt.ap().rearrange("(k p) m -> p k m", p=128)   # [128, 32, 512]
    wkt_r = wkt.ap().rearrange("(k p) m -> p k m", p=128)   # [128, 32, 128]
    wvt_r = wvt.ap().rearrange("(k p) m -> p k m", p=128)   # [128, 32, 128]
    wot_r = wot.ap().rearrange("(k p) n -> p k n", p=128)   # [128, 4, T]
    out_r = out.ap()

    with tile.TileContext(nc) as tc:
        with ExitStack() as ctx:
            persist = ctx.enter_context(tc.tile_pool(name="persist", bufs=1))
            q_sb = persist.tile([128, G, T], BF16)       # qT per head, 4MB
            k_sb = persist.tile([128, T], BF16)          # kT, 1MB
            v_sb = persist.tile([128, TT, D], BF16)      # v natural, 1MB
            wo_sb = persist.tile([128, G, T], BF16)      # woT resident, 4MB
            ones_sb = persist.tile([128, 128], BF16)

            # ---------------- phase 1: projections ----------------
            with ExitStack() as c1:
                wpool = c1.enter_context(tc.tile_pool(name="wpool", bufs=1))
                xpool = c1.enter_context(tc.tile_pool(name="xpool", bufs=12))
                vstg = c1.enter_context(tc.tile_pool(name="vstg", bufs=2))
                ps1 = c1.enter_context(tc.tile_pool(name="ps1", bufs=1, space="PSUM"))
                pstr = c1.enter_context(tc.tile_pool(name="pstr", bufs=2, space="PSUM"))

                # one tile PER k-chunk: dependency tracking is
                # tile-granular, so a single big tile would make the first
                # matmul wait for ALL 32 chunk DMAs
                wq_t = [wpool.tile([128, G * D], BF16, name=f"wq{k}")
                        for k in range(HK)]
                # wk/wv as quads of 4 k-chunks: 1KB DMA lines instead of
                # 256B, for better DMA efficiency in the bandwidth-bound
                # startup window
                wk_t = [wpool.tile([128, 4, D], BF16, name=f"wk{qi}")
                        for qi in range(HK // 4)]
                wv_t = [wpool.tile([128, 4, D], BF16, name=f"wv{qi}")
                        for qi in range(HK // 4)]
                ident = wpool.tile([128, 128], BF16)
                # alternate posts between the two free DMA queues in k
                # (= consumption) order so arrival tracks need
                posts = []
                for k in range(HK):
                    posts.append((wq_t[k], wqt_r[:, k, :]))
                    if k % 4 == 1:
                        qi = k // 4
                        posts.append((wk_t[qi], wkt_r[:, 4 * qi:4 * qi + 4, :]))
                        posts.append((wv_t[qi], wvt_r[:, 4 * qi:4 * qi + 4, :]))
                for i, (dst, src) in enumerate(posts):
                    q = nc.gpsimd if i % 2 == 0 else nc.scalar
                    q.dma_start(out=dst, in_=src)
                nc.scalar.dma_start(out=ones_sb, in_=ones.ap())
                make_identity(nc, ident)

                def v_transpose(pj, pv_st):
                    # one-j-delayed so PE never waits on the DVE staging copy
                    vt_ps = pstr.tile([128, 4, 128], BF16)
                    for tt in range(4):
                        nc.tensor.transpose(
                            vt_ps[:, tt, :], pv_st[:, tt * 128:(tt + 1) * 128],
                            ident)
                    nc.scalar.activation(
                        out=v_sb[:, 4 * pj:4 * pj + 4, :], in_=vt_ps, func=COPY)

                prev_v = None
                for j in range(NJ):
                    tsl = slice(j * 512, (j + 1) * 512)
                    q_ps = [ps1.tile([128, 512], F32, name=f"q_ps{m}")
                            for m in range(G)]
                    k_ps = ps1.tile([128, 512], F32)
                    v_ps = ps1.tile([128, 512], F32)
                    for k in range(HK):
                        x_t = xpool.tile([128, 512], BF16)
                        nc.sync.dma_start(out=x_t, in_=xt_r[:, k, tsl])
                        st = k == 0
                        sp = k == HK - 1
                        for m in range(G):
                            nc.tensor.matmul(
                                q_ps[m], wq_t[k][:, m * D:(m + 1) * D], x_t,
                                start=st, stop=sp)
                        nc.tensor.matmul(k_ps, wk_t[k // 4][:, k % 4, :], x_t,
                                         start=st, stop=sp)
                        nc.tensor.matmul(v_ps, wv_t[k // 4][:, k % 4, :], x_t,
                                         start=st, stop=sp)
                        if k == 2 and prev_v is not None:
                            v_transpose(*prev_v)
                        # prefetch wo for phase 2, gated on a j==3 x tile.
                        # The gpsimd queue posts DMAs in relaxed order, so a
                        # copy BEFORE the dma_start does not delay it; a
                        # writer-after-writer dependency on wo_sb itself does.
                        if j == 3 and k == 0:
                            nc.vector.tensor_copy(wo_sb[0:1, 0, 0:1],
                                                  x_t[0:1, 0:1])
                            for kk in range(G):
                                nc.gpsimd.dma_start(out=wo_sb[:, kk, :],
                                                    in_=wot_r[:, kk, :])
                    # split psum evacuation across ACT and DVE so the banks
                    # free up fast for the next j iteration; v first so the
                    # delayed transpose never waits on the staging copy.
                    # For the last j, evac q0/q1 first instead: phase 2's
                    # psS pool lands on the banks allocated first in this
                    # scope (q_ps0/q_ps1), so freeing those first lets the
                    # first S matmuls start ~1.4us earlier.
                    v_st = vstg.tile([128, 512], BF16)
                    if j < NJ - 1:
                        nc.vector.tensor_copy(v_st, v_ps)
                        nc.scalar.activation(out=k_sb[:, tsl], in_=k_ps,
                                             func=COPY)
                        nc.scalar.activation(out=q_sb[:, 0, tsl],
                                             in_=q_ps[0], func=COPY)
                        nc.vector.tensor_copy(q_sb[:, 1, tsl], q_ps[1])
                        nc.scalar.activation(out=q_sb[:, 2, tsl],
                                             in_=q_ps[2], func=COPY)
                        nc.vector.tensor_copy(q_sb[:, 3, tsl], q_ps[3])
                    else:
                        nc.scalar.activation(out=q_sb[:, 0, tsl],
                                             in_=q_ps[0], func=COPY)
                        nc.vector.tensor_copy(q_sb[:, 1, tsl], q_ps[1])
                        nc.vector.tensor_copy(v_st, v_ps)
                        nc.scalar.activation(out=k_sb[:, tsl], in_=k_ps,
                                             func=COPY)
                        nc.scalar.activation(out=q_sb[:, 2, tsl],
                                             in_=q_ps[2], func=COPY)
                        nc.vector.tensor_copy(q_sb[:, 3, tsl], q_ps[3])
                    prev_v = (j, v_st)
                v_transpose(*prev_v)

            # ------- phase 2: fused attention + output projection -------
            with ExitStack() as c2:
                apool = c2.enter_context(tc.tile_pool(name="apool", bufs=2))
                ppool = c2.enter_context(tc.tile_pool(name="ppool", bufs=4))
                tpool = c2.enter_context(tc.tile_pool(name="tpool", bufs=2))
                rpool = c2.enter_context(tc.tile_pool(name="rpool", bufs=2))
                opool = c2.enter_context(tc.tile_pool(name="opool", bufs=3))
                psS = c2.enter_context(tc.tile_pool(name="psS", bufs=2, space="PSUM"))
                psPV = c2.enter_context(tc.tile_pool(name="psPV", bufs=2, space="PSUM"))
                psO = c2.enter_context(tc.tile_pool(name="psO", bufs=2, space="PSUM"))

                evac_ctr = [0]

                def evac_o(o_ps, t0, n):
                    # alternate evacs over DVE/ACT by n-parity (gpsimd
                    # cannot read PSUM on TRN2). Even n (incl. the n=6
                    # filler right before a head's den tree finishes) goes
                    # to ACT so it cannot queue ahead of the tree's final
                    # adds on DVE; odd n (incl. n=7, emitted after the
                    # tree) goes to DVE.
                    o_t = opool.tile([128, 512], BF16)
                    if n % 2 == 0:
                        nc.scalar.activation(out=o_t, in_=o_ps, func=COPY)
                    else:
                        nc.vector.tensor_copy(o_t, o_ps)
                    nc.sync.dma_start(
                        out=out_r[t0:t0 + 128, n * 512:(n + 1) * 512],
                        in_=o_t)

                def outproj_nchunk(pb, pj, pa, tt2, n, o_ps=None):
                    # output projection for tq-tile tt2, H-chunk n, of chunk
                    # (pb, pj): accumulate the 4 heads in PSUM against woT.
                    t0 = pb * S + pj * 512 + tt2 * 128
                    if o_ps is None:
                        o_ps = psO.tile([128, 512], F32, name="o_ps")
                    for m in range(G):
                        nc.tensor.matmul(
                            o_ps, pa[m][:, tt2 * 128:(tt2 + 1) * 128],
                            wo_sb[:, m, n * 512:(n + 1) * 512],
                            start=(m == 0), stop=(m == G - 1))
                    evac_o(o_ps, t0, n)

                def flush_den(pend):
                    # softmax denominator of a finished head: one ones-matmul
                    # on the DVE tree sum, reciprocal, normalize into a_ch.
                    # The PSUM tile comes from the psO ring (outproj evacs
                    # release it fast) — a psS-ring tile would WAR-wait on
                    # the current head's exp(0), stalling the in-order PE.
                    acc512, pv_ps, a_t = pend
                    den_ps = psO.tile([128, 512], F32, name="o_ps")
                    nc.tensor.matmul(den_ps, ones_sb, acc512,
                                     start=True, stop=True)
                    rec_t = rpool.tile([128, 512], F32)
                    nc.vector.reciprocal_approx_fast(out=rec_t, in_=den_ps)
                    nc.vector.tensor_mul(a_t, pv_ps, rec_t)

                pending = None   # den work of the previous head
                prev = None      # a_ch of the previous (b, j)
                for b in range(B):
                    for j in range(SJ):
                        tqsl = slice(b * S + j * 512, b * S + (j + 1) * 512)
                        a_ch = [apool.tile([128, 512], BF16, name=f"a_ch{m}")
                                for m in range(G)]
                        for m in range(G):
                            pv_ps = psPV.tile([128, 512], F32,
                                              name="pv_ps")
                            p_list = []
                            t_parts = []
                            for g in range(NG):
                                s_ps = psS.tile([128, 1024], F32, name="s_ps")
                                for h in range(2):
                                    ti = b * SI + 2 * g + h
                                    nc.tensor.matmul(
                                        s_ps[:, h * 512:(h + 1) * 512],
                                        k_sb[:, ti * 128:(ti + 1) * 128],
                                        q_sb[:, m, tqsl], start=True, stop=True)
                                p_t = ppool.tile([128, 1024], BF16)
                                nc.scalar.activation(out=p_t, in_=s_ps,
                                                     func=EXP, scale=SCALE)
                                p_list.append(p_t)
                                if g % 2 == 1:
                                    tk = tpool.tile([128, 1024], BF16,
                                                    name=f"t{g // 2}")
                                    nc.vector.tensor_add(tk, p_list[g - 1],
                                                         p_list[g])
                                    t_parts.append(tk)
                                    # pre-reduce the den tree as tiles land
                                    # so only t3 -> a1024 -> acc512 remain
                                    # after exp(7): the den matmul of the
                                    # next head stalled ~0.6us on this chain
                                    if g == 3:
                                        pre = tpool.tile([128, 1024], BF16,
                                                         name="pre")
                                        nc.vector.tensor_add(
                                            pre, t_parts[0], t_parts[1])
                                    elif g == 5:
                                        pre2 = tpool.tile([128, 1024], BF16,
                                                          name="pre2")
                                        nc.vector.tensor_add(
                                            pre2, pre, t_parts[2])
                                # den flush + interleaved output projection
                                # go BEFORE the PV pair: the PE is in-order,
                                # so exp-independent work must sit ahead of
                                # the exp-dependent PV matmuls to cover the
                                # ACT latency
                                # filler schedule. m==0 keeps the baseline
                                # order (den flush at g==1) because its
                                # g==1 outproj would read the prev chunk's
                                # a_ch[3], which this very flush writes.
                                # m>=1 move the den flush to g==2 with an
                                # outproj ahead of it at g==1: the previous
                                # head's DVE tree then has ~2.5us of PE work
                                # ahead of the den matmul instead of ~0.9us
                                # (den-waits-on-tree was most of the phase-2
                                # gap time), and the outproj also covers the
                                # exp(0) latency before PV(0).
                                if m == 0:
                                    if g == 1 and pending is not None:
                                        flush_den(pending)
                                        pending = None
                                    if g >= 2 and prev is not None:
                                        outproj_nchunk(prev[0], prev[1],
                                                       prev[2], m, g - 2)
                                else:
                                    if g == 1 and prev is not None:
                                        outproj_nchunk(prev[0], prev[1],
                                                       prev[2], m, 0)
                                    if g == 2 and pending is not None:
                                        flush_den(pending)
                                        pending = None
                                    if g >= 2 and prev is not None:
                                        outproj_nchunk(prev[0], prev[1],
                                                       prev[2], m, g - 1)
                                if g >= 1:
                                    pg = p_list[g - 1]
                                    for h in range(2):
                                        ti = b * SI + 2 * (g - 1) + h
                                        nc.tensor.matmul(
                                            pv_ps, v_sb[:, ti, :],
                                            pg[:, h * 512:(h + 1) * 512],
                                            start=(g == 1 and h == 0),
                                            stop=False)
                            if m == 0 and prev is not None:
                                outproj_nchunk(prev[0], prev[1], prev[2],
                                               m, NJ - 2)
                            pg = p_list[NG - 1]
                            for h in range(2):
                                ti = b * SI + 2 * (NG - 1) + h
                                nc.tensor.matmul(
                                    pv_ps, v_sb[:, ti, :],
                                    pg[:, h * 512:(h + 1) * 512],
                                    start=False, stop=(h == 1))
                            # finish the denominator tree on DVE
                            a1024 = tpool.tile([128, 1024], BF16, name="a1024")
                            nc.vector.tensor_add(a1024, pre2, t_parts[3])
                            acc512 = tpool.tile([128, 512], BF16, name="a512")
                            nc.vector.tensor_add(acc512, a1024[:, 0:512],
                                                 a1024[:, 512:1024])
                            pending = (acc512, pv_ps, a_ch[m])
                            # post-pending filler for every head: covers the
                            # NEXT head's den matmul (m==0: tree of the prev
                            # chunk's last head) against the DVE tree latency
                            if prev is not None:
                                outproj_nchunk(prev[0], prev[1], prev[2],
                                               m, NJ - 1)
                        prev = (b, j, a_ch)
                # ---- tail: last chunk's den flush + output projection ----
                # Open two O groups with heads 0-2 first (their a_ch are
                # ready) so the in-order PE isn't stalled behind the last
                # head's den chain (DVE tree -> ones-matmul -> recip -> mul).
                # The den matmul uses a psPV-ring tile (PV is finished).
                pb, pj, pa = prev
                acc512, pv_ps, a_t = pending

                def tail_slot(i):
                    # 4-deep psum rotation for the tail: psO's 2 slots plus
                    # the (now idle) psS pool's 2 slots, so group i+4 waits
                    # on evac(i) with 3 groups of slack instead of 1
                    if i % 4 < 2:
                        return psO.tile([128, 512], F32, name="o_ps")
                    return psS.tile([128, 1024], F32, name="s_ps")[:, 0:512]

                den_ps = psPV.tile([128, 512], F32, name="pv_ps")
                nc.tensor.matmul(den_ps, ones_sb, acc512, start=True,
                                 stop=True)
                rec_t = rpool.tile([128, 512], F32)
                nc.vector.reciprocal_approx_fast(out=rec_t, in_=den_ps)
                nc.vector.tensor_mul(a_t, pv_ps, rec_t)
                ti = 0
                for tt2 in range(4):
                    for n in range(NJ):
                        outproj_nchunk(pb, pj, pa, tt2, n,
                                       o_ps=tail_slot(ti))
                        ti += 1
    nc.compile()
    return nc


_NC_CACHE = None


def _get_nc():
    global _NC_CACHE
    if _NC_CACHE is None:
        _NC_CACHE = build_nc()
    return _NC_CACHE


def make_in_maps(x, wq, wk, wv, wo):
    xt = np.ascontiguousarray(x.reshape(T, H).T).astype(bfloat16)
    wqb = wq.astype(bfloat16)
    wkb = wk.astype(bfloat16)
    wvb = wv.astype(bfloat16)
    wob = wo.astype(bfloat16)
    ones = np.ones((128, 128), dtype=bfloat16)
    in_maps = []
    for c in range(NCORES):
        qsl = slice(c * G * D, (c + 1) * G * D)
        ksl = slice(c * D, (c + 1) * D)
        in_maps.append({
            "xt": xt,
            "wqt": np.ascontiguousarray(wqb[qsl, :].T),
            "wkt": np.ascontiguousarray(wkb[ksl, :].T),
            "wvt": np.ascontiguousarray(wvb[ksl, :].T),
            "wot": np.ascontiguousarray(wob[:, qsl].T),
            "ones": ones,
        })
    return in_maps


def kernel(x, wq, wk, wv, wo, **run_kwargs):
    nc = _get_nc()
    in_maps = make_in_maps(np.asarray(x, dtype=np.float32),
                           np.asarray(wq, dtype=np.float32),
                           np.asarray(wk, dtype=np.float32),
                           np.asarray(wv, dtype=np.float32),
                           np.asarray(wo, dtype=np.float32))
    res = run_bass_kernel_spmd(nc, in_maps, core_ids=list(range(NCORES)),
                               **run_kwargs)
    acc = np.zeros((T, H), dtype=np.float32)
    for c in range(NCORES):
        acc += res.results[c]["out"].astype(np.float32)
    out = acc.reshape(B, S, H)
    if run_kwargs:
        return out, res
    return out



# revision 37
# speedup vs baseline: 1.2036x; 1.0117x over previous
"""Trainium2 Bass kernel for Llama GQA attention (no mask), 8-way tensor
parallel over KV heads.

Problem shapes (hardcoded):
  x  (2, 2048, 4096) f32
  wq (4096, 4096), wk (1024, 4096), wv (1024, 4096), wo (4096, 4096) f32
  NUM_HEADS=32, NUM_KV_HEADS=8, HEAD_DIM=128, GQA group g=4

Sharding: core c owns KV head c (4 Q heads). x replicated (pre-transposed
to xT on host), wq/wk/wv sharded on output dim (pre-transposed host-side),
wo sharded on input dim. Each core computes a partial (4096, 4096) output
(its heads' contribution through wo); host sums the 8 partials in fp32.

All tensors are bf16 (PSUM accumulation fp32): same PE rate as fp32r
(1 col/cycle) but half the DMA/SBUF traffic, which removes the phase-1
x-feed stalls the fp32 version had.

Structure:
  phase 1: q/k/v projections. Weight chunk k (wq+wk+wv) DMA'd as
    per-k tiles (dependency tracking is tile-granular) alternating
    gpsimd/scalar queues; x tiles on the sync queue (12-deep ring).
    wo prefetch is WAR-gated on a j==3 x tile so the run-ahead DMA
    queues can't flood the startup window. vT -> v via PE transposes.
    PSUM evacuations split ACT/DVE, k first (phase-2 boundary).
  phase 2 (fused attention + output projection, software-pipelined):
    per (batch, tq-chunk, head): scores transposed ST = kT_tile.T @ qT
    into [128,1024] PSUM (2 k-tiles per matmul pair), batched exp ->
    p (bf16). Softmax denominator = DVE pairwise-tree sum of the 16 p
    tiles + ONE ones-matmul per head (instead of 16 PE den matmuls).
    The PE is in-order, so per g the emission is: S-pair(g), then
    exp-independent filler (den flush of the previous head / one
    output-projection H-chunk of the previous (b,chunk)), then
    PV-pair(g-1) — the filler covers the ACT exp latency. For heads
    m>=1 the den flush sits at g==2 behind an outproj at g==1, giving
    the previous head's DVE tree ~2.5us of PE cover (den-waits-on-tree
    was most of the phase-2 gap time); m==0 keeps the flush at g==1
    because its g==1 outproj would read the a_ch[3] that this flush
    writes. Every head ends with an outproj after its tree so the next
    den matmul never leads the tree. The last (b,chunk)'s output
    projection drains in a tail block with a 4-deep PSUM rotation
    (psO's 2 slots + the idle psS pool's 2).

    Measured constraints (this session): fp8e4m3 DoubleRow = 2x flops
    only (216ns for K=256 x 512 cols, same as bf16 K=128) and any
    single-fp8 operand costs 3e-2..7e-2 rel err (budget 2e-2), so fp8
    cannot beat the 766us bf16 PE floor here. DMA aggregate ~244GB/s
    regardless of line size; the j=0 window (weights 6MB + x 4MB in
    41us) is at capacity, so ~12us of early PE stalls are structural.
    GPSIMD cannot read PSUM. Splitting PSUM accumulation groups with
    interleaved matmuls costs ~0.6us per resume - avoid open groups.
"""

import sys
from contextlib import ExitStack

import numpy as np
from ml_dtypes import bfloat16

sys.path.insert(0, "/opt/trn_rl_repo")

import concourse.bass as bass  # noqa: E402
import concourse.tile as tile  # noqa: E402
from concourse import bacc, mybir  # noqa: E402
from concourse.bass_utils import run_bass_kernel_spmd  # noqa: E402
from concourse.masks import make_identity  # noqa: E402

NCORES = 8
B, S, H = 2, 2048, 4096
T = B * S                      # 4096 flattened tokens
D = 128                        # head dim
G = 4                          # q heads per core (GQA group)
HK = 32                        # h k-tiles (4096 / 128)
TT = T // 128                  # 32 token tiles
NJ = T // 512                  # 8 token chunks of 512
SJ = S // 512                  # 4 tq chunks per batch
SI = S // 128                  # 16 tk tiles per batch
NG = SI // 2                   # 8 k-tile pairs per batch
SCALE = float(1.0 / np.sqrt(D))

F32 = mybir.dt.float32
BF16 = mybir.dt.bfloat16
COPY = mybir.ActivationFunctionType.Copy
EXP = mybir.ActivationFunctionType.Exp


def build_nc():
    nc = bacc.Bacc("TRN2", target_bir_lowering=False, debug=False,
                   enable_asserts=True, num_devices=NCORES)
    xt = nc.declare_dram_parameter("xt", [H, T], BF16, isOutput=False)
    wqt = nc.declare_dram_parameter("wqt", [H, G * D], BF16, isOutput=False)
    wkt = nc.declare_dram_parameter("wkt", [H, D], BF16, isOutput=False)
    wvt = nc.declare_dram_parameter("wvt", [H, D], BF16, isOutput=False)
    wot = nc.declare_dram_parameter("wot", [G * D, H], BF16, isOutput=False)
    ones = nc.declare_dram_parameter("ones", [128, 128], BF16, isOutput=False)
    out = nc.declare_dram_parameter("out", [T, H], BF16, isOutput=True)

    xt_r = xt.ap().rearrange("(k p) t -> p k t", p=128)     # [128, 32, T]
    wqt_r = wqt.ap().rearrange("(k p) m -> p k m", p=128)   # [128, 32, 512]
    wkt_r = wkt.ap().rearrange("(k p) m -> p k m", p=128)   # [128, 32, 128]
    wvt_r = wvt.ap().rearrange("(k p) m -> p k m", p=128)   # [128, 32, 128]
    wot_r = wot.ap().rearrange("(k p) n -> p k n", p=128)   # [128, 4, T]
    out_r = out.ap()

    with tile.TileContext(nc) as tc:
        with ExitStack() as ctx:
            persist = ctx.enter_context(tc.tile_pool(name="persist", bufs=1))
            q_sb = persist.tile([128, G, T], BF16)       # qT per head, 4MB
            k_sb = persist.tile([128, T], BF16)          # kT, 1MB
            v_sb = persist.tile([128, TT, D], BF16)      # v natural, 1MB
            wo_sb = persist.tile([128, G, T], BF16)      # woT resident, 4MB
            ones_sb = persist.tile([128, 128], BF16)

            # ---------------- phase 1: projections ----------------
            with ExitStack() as c1:
                wpool = c1.enter_context(tc.tile_pool(name="wpool", bufs=1))
                xpool = c1.enter_context(tc.tile_pool(name="xpool", bufs=12))
                vstg = c1.enter_context(tc.tile_pool(name="vstg", bufs=2))
                ps1 = c1.enter_context(tc.tile_pool(name="ps1", bufs=1, space="PSUM"))
                pstr = c1.enter_context(tc.tile_pool(name="pstr", bufs=2, space="PSUM"))

                # one tile PER k-chunk: dependency tracking is
                # tile-granular, so a single big tile would make the first
                # matmul wait for ALL 32 chunk DMAs
                wq_t = [wpool.tile([128, G * D], BF16, name=f"wq{k}")
                        for k in range(HK)]
                # wk/wv as quads of 4 k-chunks: 1KB DMA lines instead of
                # 256B, for better DMA efficiency in the bandwidth-bound
                # startup window
                wk_t = [wpool.tile([128, 4, D], BF16, name=f"wk{qi}")
                        for qi in range(HK // 4)]
                wv_t = [wpool.tile([128, 4, D], BF16, name=f"wv{qi}")
                        for qi in range(HK // 4)]
                ident = wpool.tile([128, 128], BF16)
                # alternate posts between the two free DMA queues in k
                # (= consumption) order so arrival tracks need
                posts = []
                for k in range(HK):
                    posts.append((wq_t[k], wqt_r[:, k, :]))
                    if k % 4 == 1:
                        qi = k // 4
                        posts.append((wk_t[qi], wkt_r[:, 4 * qi:4 * qi + 4, :]))
                        posts.append((wv_t[qi], wvt_r[:, 4 * qi:4 * qi + 4, :]))
                for i, (dst, src) in enumerate(posts):
                    q = nc.gpsimd if i % 2 == 0 else nc.scalar
                    q.dma_start(out=dst, in_=src)
                nc.scalar.dma_start(out=ones_sb, in_=ones.ap())
                make_identity(nc, ident)

                def v_transpose(pj, pv_st):
                    # one-j-delayed so PE never waits on the DVE staging copy
                    vt_ps = pstr.tile([128, 4, 128], BF16)
                    for tt in range(4):
                        nc.tensor.transpose(
                            vt_ps[:, tt, :], pv_st[:, tt * 128:(tt + 1) * 128],
                            ident)
                    nc.scalar.activation(
                        out=v_sb[:, 4 * pj:4 * pj + 4, :], in_=vt_ps, func=COPY)

                prev_v = None
                for j in range(NJ):
                    tsl = slice(j * 512, (j + 1) * 512)
                    q_ps = [ps1.tile([128, 512], F32, name=f"q_ps{m}")
                            for m in range(G)]
                    k_ps = ps1.tile([128, 512], F32)
                    v_ps = ps1.tile([128, 512], F32)
                    for k in range(HK):
                        x_t = xpool.tile([128, 512], BF16)
                        nc.sync.dma_start(out=x_t, in_=xt_r[:, k, tsl])
                        st = k == 0
                        sp = k == HK - 1
                        for m in range(G):
                            nc.tensor.matmul(
                                q_ps[m], wq_t[k][:, m * D:(m + 1) * D], x_t,
                                start=st, stop=sp)
                        nc.tensor.matmul(k_ps, wk_t[k // 4][:, k % 4, :], x_t,
                                         start=st, stop=sp)
                        nc.tensor.matmul(v_ps, wv_t[k // 4][:, k % 4, :], x_t,
                                         start=st, stop=sp)
                        if k == 2 and prev_v is not None:
                            v_transpose(*prev_v)
                        # prefetch wo for phase 2, gated on a j==3 x tile.
                        # The gpsimd queue posts DMAs in relaxed order, so a
                        # copy BEFORE the dma_start does not delay it; a
                        # writer-after-writer dependency on wo_sb itself does.
                        if j == 3 and k == 0:
                            nc.vector.tensor_copy(wo_sb[0:1, 0, 0:1],
                                                  x_t[0:1, 0:1])
                            for kk in range(G):
                                nc.gpsimd.dma_start(out=wo_sb[:, kk, :],
                                                    in_=wot_r[:, kk, :])
                    # split psum evacuation across ACT and DVE so the banks
                    # free up fast for the next j iteration; v first so the
                    # delayed transpose never waits on the staging copy.
                    # For the last j, evac q0/q1 first instead: phase 2's
                    # psS pool lands on the banks allocated first in this
                    # scope (q_ps0/q_ps1), so freeing those first lets the
                    # first S matmuls start ~1.4us earlier.
                    v_st = vstg.tile([128, 512], BF16)
                    if j < NJ - 1:
                        nc.vector.tensor_copy(v_st, v_ps)
                        nc.scalar.activation(out=k_sb[:, tsl], in_=k_ps,
                                             func=COPY)
                        nc.scalar.activation(out=q_sb[:, 0, tsl],
                                             in_=q_ps[0], func=COPY)
                        nc.vector.tensor_copy(q_sb[:, 1, tsl], q_ps[1])
                        nc.scalar.activation(out=q_sb[:, 2, tsl],
                                             in_=q_ps[2], func=COPY)
                        nc.vector.tensor_copy(q_sb[:, 3, tsl], q_ps[3])
                    else:
                        nc.scalar.activation(out=q_sb[:, 0, tsl],
                                             in_=q_ps[0], func=COPY)
                        nc.vector.tensor_copy(q_sb[:, 1, tsl], q_ps[1])
                        nc.vector.tensor_copy(v_st, v_ps)
                        nc.scalar.activation(out=k_sb[:, tsl], in_=k_ps,
                                             func=COPY)
                        nc.scalar.activation(out=q_sb[:, 2, tsl],
                                             in_=q_ps[2], func=COPY)
                        nc.vector.tensor_copy(q_sb[:, 3, tsl], q_ps[3])
                    prev_v = (j, v_st)
                v_transpose(*prev_v)

            # ------- phase 2: fused attention + output projection -------
            with ExitStack() as c2:
                apool = c2.enter_context(tc.tile_pool(name="apool", bufs=2))
                ppool = c2.enter_context(tc.tile_pool(name="ppool", bufs=4))
                tpool = c2.enter_context(tc.tile_pool(name="tpool", bufs=2))
                rpool = c2.enter_context(tc.tile_pool(name="rpool", bufs=2))
                opool = c2.enter_context(tc.tile_pool(name="opool", bufs=8))
                psS = c2.enter_context(tc.tile_pool(name="psS", bufs=2, space="PSUM"))
                psPV = c2.enter_context(tc.tile_pool(name="psPV", bufs=2, space="PSUM"))
                psO = c2.enter_context(tc.tile_pool(name="psO", bufs=2, space="PSUM"))

                evac_ctr = [0]

                def evac_o(o_ps, t0, n):
                    # alternate evacs over DVE/ACT by n-parity (gpsimd
                    # cannot read PSUM on TRN2). Even n (incl. the n=6
                    # filler right before a head's den tree finishes) goes
                    # to ACT so it cannot queue ahead of the tree's final
                    # adds on DVE; odd n (incl. n=7, emitted after the
                    # tree) goes to DVE.
                    o_t = opool.tile([128, 512], BF16)
                    if n % 2 == 0:
                        nc.scalar.activation(out=o_t, in_=o_ps, func=COPY)
                    else:
                        nc.vector.tensor_copy(o_t, o_ps)
                    nc.sync.dma_start(
                        out=out_r[t0:t0 + 128, n * 512:(n + 1) * 512],
                        in_=o_t)

                def outproj_nchunk(pb, pj, pa, tt2, n, o_ps=None):
                    # output projection for tq-tile tt2, H-chunk n, of chunk
                    # (pb, pj): accumulate the 4 heads in PSUM against woT.
                    t0 = pb * S + pj * 512 + tt2 * 128
                    if o_ps is None:
                        o_ps = psO.tile([128, 512], F32, name="o_ps")
                    for m in range(G):
                        nc.tensor.matmul(
                            o_ps, pa[m][:, tt2 * 128:(tt2 + 1) * 128],
                            wo_sb[:, m, n * 512:(n + 1) * 512],
                            start=(m == 0), stop=(m == G - 1))
                    evac_o(o_ps, t0, n)

                def flush_den(pend):
                    # softmax denominator of a finished head: one ones-matmul
                    # on the DVE tree sum, reciprocal, normalize into a_ch.
                    # The PSUM tile comes from the psO ring (outproj evacs
                    # release it fast) — a psS-ring tile would WAR-wait on
                    # the current head's exp(0), stalling the in-order PE.
                    acc512, pv_ps, a_t = pend
                    den_ps = psO.tile([128, 512], F32, name="o_ps")
                    nc.tensor.matmul(den_ps, ones_sb, acc512,
                                     start=True, stop=True)
                    rec_t = rpool.tile([128, 512], F32)
                    nc.vector.reciprocal_approx_fast(out=rec_t, in_=den_ps)
                    nc.vector.tensor_mul(a_t, pv_ps, rec_t)

                pending = None   # den work of the previous head
                prev = None      # a_ch of the previous (b, j)
                for b in range(B):
                    for j in range(SJ):
                        tqsl = slice(b * S + j * 512, b * S + (j + 1) * 512)
                        a_ch = [apool.tile([128, 512], BF16, name=f"a_ch{m}")
                                for m in range(G)]
                        for m in range(G):
                            pv_ps = psPV.tile([128, 512], F32,
                                              name="pv_ps")
                            p_list = []
                            t_parts = []
                            for g in range(NG):
                                s_ps = psS.tile([128, 1024], F32, name="s_ps")
                                for h in range(2):
                                    ti = b * SI + 2 * g + h
                                    nc.tensor.matmul(
                                        s_ps[:, h * 512:(h + 1) * 512],
                                        k_sb[:, ti * 128:(ti + 1) * 128],
                                        q_sb[:, m, tqsl], start=True, stop=True)
                                p_t = ppool.tile([128, 1024], BF16)
                                nc.scalar.activation(out=p_t, in_=s_ps,
                                                     func=EXP, scale=SCALE)
                                p_list.append(p_t)
                                if g % 2 == 1:
                                    tk = tpool.tile([128, 1024], BF16,
                                                    name=f"t{g // 2}")
                                    nc.vector.tensor_add(tk, p_list[g - 1],
                                                         p_list[g])
                                    t_parts.append(tk)
                                    # pre-reduce the den tree as tiles land
                                    # so only t3 -> a1024 -> acc512 remain
                                    # after exp(7): the den matmul of the
                                    # next head stalled ~0.6us on this chain
                                    if g == 3:
                                        pre = tpool.tile([128, 1024], BF16,
                                                         name="pre")
                                        nc.vector.tensor_add(
                                            pre, t_parts[0], t_parts[1])
                                    elif g == 5:
                                        pre2 = tpool.tile([128, 1024], BF16,
                                                          name="pre2")
                                        nc.vector.tensor_add(
                                            pre2, pre, t_parts[2])
                                # den flush + interleaved output projection
                                # go BEFORE the PV pair: the PE is in-order,
                                # so exp-independent work must sit ahead of
                                # the exp-dependent PV matmuls to cover the
                                # ACT latency
                                # filler schedule. m==0 keeps the baseline
                                # order (den flush at g==1) because its
                                # g==1 outproj would read the prev chunk's
                                # a_ch[3], which this very flush writes.
                                # m>=1 move the den flush to g==2 with an
                                # outproj ahead of it at g==1: the previous
                                # head's DVE tree then has ~2.5us of PE work
                                # ahead of the den matmul instead of ~0.9us
                                # (den-waits-on-tree was most of the phase-2
                                # gap time), and the outproj also covers the
                                # exp(0) latency before PV(0).
                                if m == 0:
                                    if g == 1 and pending is not None:
                                        flush_den(pending)
                                        pending = None
                                    if g >= 2 and prev is not None:
                                        outproj_nchunk(prev[0], prev[1],
                                                       prev[2], m, g - 2)
                                else:
                                    if g == 1 and prev is not None:
                                        outproj_nchunk(prev[0], prev[1],
                                                       prev[2], m, 0)
                                    if g == 2 and pending is not None:
                                        flush_den(pending)
                                        pending = None
                                    if g >= 2 and prev is not None:
                                        outproj_nchunk(prev[0], prev[1],
                                                       prev[2], m, g - 1)
                                if g >= 1:
                                    pg = p_list[g - 1]
                                    for h in range(2):
                                        ti = b * SI + 2 * (g - 1) + h
                                        nc.tensor.matmul(
                                            pv_ps, v_sb[:, ti, :],
                                            pg[:, h * 512:(h + 1) * 512],
                                            start=(g == 1 and h == 0),
                                            stop=False)
                            if m == 0 and prev is not None:
                                outproj_nchunk(prev[0], prev[1], prev[2],
                                               m, NJ - 2)
                            pg = p_list[NG - 1]
                            for h in range(2):
                                ti = b * SI + 2 * (NG - 1) + h
                                nc.tensor.matmul(
                                    pv_ps, v_sb[:, ti, :],
                                    pg[:, h * 512:(h + 1) * 512],
                                    start=False, stop=(h == 1))
                            # finish the denominator tree on DVE
                            a1024 = tpool.tile([128, 1024], BF16, name="a1024")
                            nc.vector.tensor_add(a1024, pre2, t_parts[3])
                            acc512 = tpool.tile([128, 512], BF16, name="a512")
                            nc.vector.tensor_add(acc512, a1024[:, 0:512],
                                                 a1024[:, 512:1024])
                            pending = (acc512, pv_ps, a_ch[m])
                            # post-pending filler for every head: covers the
                            # NEXT head's den matmul (m==0: tree of the prev
                            # chunk's last head) against the DVE tree latency
                            if prev is not None:
                                outproj_nchunk(prev[0], prev[1], prev[2],
                                               m, NJ - 1)
                        prev = (b, j, a_ch)
                # ---- tail: last chunk's den flush + output projection ----
                # Open two O groups with heads 0-2 first (their a_ch are
                # ready) so the in-order PE isn't stalled behind the last
                # head's den chain (DVE tree -> ones-matmul -> recip -> mul).
                # The den matmul uses a psPV-ring tile (PV is finished).
                pb, pj, pa = prev
                acc512, pv_ps, a_t = pending

                def tail_slot(i):
                    # 4-deep psum rotation for the tail: psO's 2 slots plus
                    # the (now idle) psS pool's 2 slots, so group i+4 waits
                    # on evac(i) with 3 groups of slack instead of 1
                    if i % 4 < 2:
                        return psO.tile([128, 512], F32, name="o_ps")
                    return psS.tile([128, 1024], F32, name="s_ps")[:, 0:512]

                den_ps = psPV.tile([128, 512], F32, name="pv_ps")
                nc.tensor.matmul(den_ps, ones_sb, acc512, start=True,
                                 stop=True)
                rec_t = rpool.tile([128, 512], F32)
                nc.vector.reciprocal_approx_fast(out=rec_t, in_=den_ps)
                nc.vector.tensor_mul(a_t, pv_ps, rec_t)
                ti = 0
                for tt2 in range(4):
                    for n in range(NJ):
                        outproj_nchunk(pb, pj, pa, tt2, n,
                                       o_ps=tail_slot(ti))
                        ti += 1
    nc.compile()
    return nc


_NC_CACHE = None


def _get_nc():
    global _NC_CACHE
    if _NC_CACHE is None:
        _NC_CACHE = build_nc()
    return _NC_CACHE


def make_in_maps(x, wq, wk, wv, wo):
    xt = np.ascontiguousarray(x.reshape(T, H).T).astype(bfloat16)
    wqb = wq.astype(bfloat16)
    wkb = wk.astype(bfloat16)
    wvb = wv.astype(bfloat16)
    wob = wo.astype(bfloat16)
    ones = np.ones((128, 128), dtype=bfloat16)
    in_maps = []
    for c in range(NCORES):
        qsl = slice(c * G * D, (c + 1) * G * D)
        ksl = slice(c * D, (c + 1) * D)
        in_maps.append({
            "xt": xt,
            "wqt": np.ascontiguousarray(wqb[qsl, :].T),
            "wkt": np.ascontiguousarray(wkb[ksl, :].T),
            "wvt": np.ascontiguousarray(wvb[ksl, :].T),
            "wot": np.ascontiguousarray(wob[:, qsl].T),
            "ones": ones,
        })
    return in_maps


def kernel(x, wq, wk, wv, wo, **run_kwargs):
    nc = _get_nc()
    in_maps = make_in_maps(np.asarray(x, dtype=np.float32),
                           np.asarray(wq, dtype=np.float32),
                           np.asarray(wk, dtype=np.float32),
                           np.asarray(wv, dtype=np.float32),
                           np.asarray(wo, dtype=np.float32))
    res = run_bass_kernel_spmd(nc, in_maps, core_ids=list(range(NCORES)),
                               **run_kwargs)
    acc = np.zeros((T, H), dtype=np.float32)
    for c in range(NCORES):
        acc += res.results[c]["out"].astype(np.float32)
    out = acc.reshape(B, S, H)
    if run_kwargs:
        return out, res
    return out



# revision 38
# speedup vs baseline: 1.2076x; 1.0032x over previous
"""Trainium2 Bass kernel for Llama GQA attention (no mask), 8-way tensor
parallel over KV heads.

Problem shapes (hardcoded):
  x  (2, 2048, 4096) f32
  wq (4096, 4096), wk (1024, 4096), wv (1024, 4096), wo (4096, 4096) f32
  NUM_HEADS=32, NUM_KV_HEADS=8, HEAD_DIM=128, GQA group g=4

Sharding: core c owns KV head c (4 Q heads). x replicated (pre-transposed
to xT on host), wq/wk/wv sharded on output dim (pre-transposed host-side),
wo sharded on input dim. Each core computes a partial (4096, 4096) output
(its heads' contribution through wo); host sums the 8 partials in fp32.

All tensors are bf16 (PSUM accumulation fp32): same PE rate as fp32r
(1 col/cycle) but half the DMA/SBUF traffic, which removes the phase-1
x-feed stalls the fp32 version had.

Structure:
  phase 1: q/k/v projections. Weight chunk k (wq+wk+wv) DMA'd as
    per-k tiles (dependency tracking is tile-granular) alternating
    gpsimd/scalar queues; x tiles on the sync queue (12-deep ring).
    wo prefetch is WAR-gated on a j==3 x tile so the run-ahead DMA
    queues can't flood the startup window. vT -> v via PE transposes.
    PSUM evacuations split ACT/DVE, k first (phase-2 boundary).
  phase 2 (fused attention + output projection, software-pipelined):
    per (batch, tq-chunk, head): scores transposed ST = kT_tile.T @ qT
    into [128,1024] PSUM (2 k-tiles per matmul pair), batched exp ->
    p (bf16). Softmax denominator = DVE pairwise-tree sum of the 16 p
    tiles + ONE ones-matmul per head (instead of 16 PE den matmuls).
    The PE is in-order, so per g the emission is: S-pair(g), then
    exp-independent filler (den flush of the previous head / one
    output-projection H-chunk of the previous (b,chunk)), then
    PV-pair(g-1) — the filler covers the ACT exp latency. For heads
    m>=1 the den flush sits at g==2 behind an outproj at g==1, giving
    the previous head's DVE tree ~2.5us of PE cover (den-waits-on-tree
    was most of the phase-2 gap time); m==0 keeps the flush at g==1
    because its g==1 outproj would read the a_ch[3] that this flush
    writes. Every head ends with an outproj after its tree so the next
    den matmul never leads the tree. The last (b,chunk)'s output
    projection drains in a tail block with a 4-deep PSUM rotation
    (psO's 2 slots + the idle psS pool's 2).

    Measured constraints (this session): fp8e4m3 DoubleRow = 2x flops
    only (216ns for K=256 x 512 cols, same as bf16 K=128) and any
    single-fp8 operand costs 3e-2..7e-2 rel err (budget 2e-2), so fp8
    cannot beat the 766us bf16 PE floor here. DMA aggregate ~244GB/s
    regardless of line size; the j=0 window (weights 6MB + x 4MB in
    41us) is at capacity, so ~12us of early PE stalls are structural.
    GPSIMD cannot read PSUM. Splitting PSUM accumulation groups with
    interleaved matmuls costs ~0.6us per resume - avoid open groups.
"""

import sys
from contextlib import ExitStack

import numpy as np
from ml_dtypes import bfloat16

sys.path.insert(0, "/opt/trn_rl_repo")

import concourse.bass as bass  # noqa: E402
import concourse.tile as tile  # noqa: E402
from concourse import bacc, mybir  # noqa: E402
from concourse.bass_utils import run_bass_kernel_spmd  # noqa: E402
from concourse.masks import make_identity  # noqa: E402

NCORES = 8
B, S, H = 2, 2048, 4096
T = B * S                      # 4096 flattened tokens
D = 128                        # head dim
G = 4                          # q heads per core (GQA group)
HK = 32                        # h k-tiles (4096 / 128)
TT = T // 128                  # 32 token tiles
NJ = T // 512                  # 8 token chunks of 512
SJ = S // 512                  # 4 tq chunks per batch
SI = S // 128                  # 16 tk tiles per batch
NG = SI // 2                   # 8 k-tile pairs per batch
SCALE = float(1.0 / np.sqrt(D))

F32 = mybir.dt.float32
BF16 = mybir.dt.bfloat16
COPY = mybir.ActivationFunctionType.Copy
EXP = mybir.ActivationFunctionType.Exp


def build_nc():
    nc = bacc.Bacc("TRN2", target_bir_lowering=False, debug=False,
                   enable_asserts=True, num_devices=NCORES)
    xt = nc.declare_dram_parameter("xt", [H, T], BF16, isOutput=False)
    wqt = nc.declare_dram_parameter("wqt", [H, G * D], BF16, isOutput=False)
    wkt = nc.declare_dram_parameter("wkt", [H, D], BF16, isOutput=False)
    wvt = nc.declare_dram_parameter("wvt", [H, D], BF16, isOutput=False)
    wot = nc.declare_dram_parameter("wot", [G * D, H], BF16, isOutput=False)
    ones = nc.declare_dram_parameter("ones", [128, 128], BF16, isOutput=False)
    out = nc.declare_dram_parameter("out", [T, H], BF16, isOutput=True)

    xt_r = xt.ap().rearrange("(k p) t -> p k t", p=128)     # [128, 32, T]
    wqt_r = wqt.ap().rearrange("(k p) m -> p k m", p=128)   # [128, 32, 512]
    wkt_r = wkt.ap().rearrange("(k p) m -> p k m", p=128)   # [128, 32, 128]
    wvt_r = wvt.ap().rearrange("(k p) m -> p k m", p=128)   # [128, 32, 128]
    wot_r = wot.ap().rearrange("(k p) n -> p k n", p=128)   # [128, 4, T]
    out_r = out.ap()

    with tile.TileContext(nc) as tc:
        with ExitStack() as ctx:
            persist = ctx.enter_context(tc.tile_pool(name="persist", bufs=1))
            q_sb = persist.tile([128, G, T], BF16)       # qT per head, 4MB
            k_sb = persist.tile([128, T], BF16)          # kT, 1MB
            v_sb = persist.tile([128, TT, D], BF16)      # v natural, 1MB
            wo_sb = persist.tile([128, G, T], BF16)      # woT resident, 4MB
            ones_sb = persist.tile([128, 128], BF16)

            # ---------------- phase 1: projections ----------------
            with ExitStack() as c1:
                wpool = c1.enter_context(tc.tile_pool(name="wpool", bufs=1))
                xpool = c1.enter_context(tc.tile_pool(name="xpool", bufs=20))
                vstg = c1.enter_context(tc.tile_pool(name="vstg", bufs=2))
                ps1 = c1.enter_context(tc.tile_pool(name="ps1", bufs=1, space="PSUM"))
                pstr = c1.enter_context(tc.tile_pool(name="pstr", bufs=2, space="PSUM"))

                # one tile PER k-chunk: dependency tracking is
                # tile-granular, so a single big tile would make the first
                # matmul wait for ALL 32 chunk DMAs
                wq_t = [wpool.tile([128, G * D], BF16, name=f"wq{k}")
                        for k in range(HK)]
                # wk/wv as quads of 4 k-chunks: 1KB DMA lines instead of
                # 256B, for better DMA efficiency in the bandwidth-bound
                # startup window
                wk_t = [wpool.tile([128, 4, D], BF16, name=f"wk{qi}")
                        for qi in range(HK // 4)]
                wv_t = [wpool.tile([128, 4, D], BF16, name=f"wv{qi}")
                        for qi in range(HK // 4)]
                ident = wpool.tile([128, 128], BF16)
                # alternate posts between the two free DMA queues in k
                # (= consumption) order so arrival tracks need
                posts = []
                for k in range(HK):
                    posts.append((wq_t[k], wqt_r[:, k, :]))
                    if k % 4 == 1:
                        qi = k // 4
                        posts.append((wk_t[qi], wkt_r[:, 4 * qi:4 * qi + 4, :]))
                        posts.append((wv_t[qi], wvt_r[:, 4 * qi:4 * qi + 4, :]))
                for i, (dst, src) in enumerate(posts):
                    q = nc.gpsimd if i % 2 == 0 else nc.scalar
                    q.dma_start(out=dst, in_=src)
                nc.scalar.dma_start(out=ones_sb, in_=ones.ap())
                make_identity(nc, ident)

                def v_transpose(pj, pv_st):
                    # one-j-delayed so PE never waits on the DVE staging copy
                    vt_ps = pstr.tile([128, 4, 128], BF16)
                    for tt in range(4):
                        nc.tensor.transpose(
                            vt_ps[:, tt, :], pv_st[:, tt * 128:(tt + 1) * 128],
                            ident)
                    nc.scalar.activation(
                        out=v_sb[:, 4 * pj:4 * pj + 4, :], in_=vt_ps, func=COPY)

                prev_v = None
                for j in range(NJ):
                    tsl = slice(j * 512, (j + 1) * 512)
                    q_ps = [ps1.tile([128, 512], F32, name=f"q_ps{m}")
                            for m in range(G)]
                    k_ps = ps1.tile([128, 512], F32)
                    v_ps = ps1.tile([128, 512], F32)
                    for k in range(HK):
                        x_t = xpool.tile([128, 512], BF16)
                        nc.sync.dma_start(out=x_t, in_=xt_r[:, k, tsl])
                        st = k == 0
                        sp = k == HK - 1
                        for m in range(G):
                            nc.tensor.matmul(
                                q_ps[m], wq_t[k][:, m * D:(m + 1) * D], x_t,
                                start=st, stop=sp)
                        nc.tensor.matmul(k_ps, wk_t[k // 4][:, k % 4, :], x_t,
                                         start=st, stop=sp)
                        nc.tensor.matmul(v_ps, wv_t[k // 4][:, k % 4, :], x_t,
                                         start=st, stop=sp)
                        if k == 2 and prev_v is not None:
                            v_transpose(*prev_v)
                        # prefetch wo for phase 2, gated on a j==3 x tile.
                        # The gpsimd queue posts DMAs in relaxed order, so a
                        # copy BEFORE the dma_start does not delay it; a
                        # writer-after-writer dependency on wo_sb itself does.
                        if j == 3 and k == 0:
                            nc.vector.tensor_copy(wo_sb[0:1, 0, 0:1],
                                                  x_t[0:1, 0:1])
                            for kk in range(G):
                                nc.gpsimd.dma_start(out=wo_sb[:, kk, :],
                                                    in_=wot_r[:, kk, :])
                    # split psum evacuation across ACT and DVE so the banks
                    # free up fast for the next j iteration; v first so the
                    # delayed transpose never waits on the staging copy.
                    # For the last j, evac q0/q1 first instead: phase 2's
                    # psS pool lands on the banks allocated first in this
                    # scope (q_ps0/q_ps1), so freeing those first lets the
                    # first S matmuls start ~1.4us earlier.
                    v_st = vstg.tile([128, 512], BF16)
                    if j < NJ - 1:
                        nc.vector.tensor_copy(v_st, v_ps)
                        nc.scalar.activation(out=k_sb[:, tsl], in_=k_ps,
                                             func=COPY)
                        nc.scalar.activation(out=q_sb[:, 0, tsl],
                                             in_=q_ps[0], func=COPY)
                        nc.vector.tensor_copy(q_sb[:, 1, tsl], q_ps[1])
                        nc.scalar.activation(out=q_sb[:, 2, tsl],
                                             in_=q_ps[2], func=COPY)
                        nc.vector.tensor_copy(q_sb[:, 3, tsl], q_ps[3])
                    else:
                        nc.scalar.activation(out=q_sb[:, 0, tsl],
                                             in_=q_ps[0], func=COPY)
                        nc.vector.tensor_copy(q_sb[:, 1, tsl], q_ps[1])
                        nc.vector.tensor_copy(v_st, v_ps)
                        nc.scalar.activation(out=k_sb[:, tsl], in_=k_ps,
                                             func=COPY)
                        nc.scalar.activation(out=q_sb[:, 2, tsl],
                                             in_=q_ps[2], func=COPY)
                        nc.vector.tensor_copy(q_sb[:, 3, tsl], q_ps[3])
                    prev_v = (j, v_st)
                v_transpose(*prev_v)

            # ------- phase 2: fused attention + output projection -------
            with ExitStack() as c2:
                apool = c2.enter_context(tc.tile_pool(name="apool", bufs=2))
                ppool = c2.enter_context(tc.tile_pool(name="ppool", bufs=6))
                tpool = c2.enter_context(tc.tile_pool(name="tpool", bufs=2))
                rpool = c2.enter_context(tc.tile_pool(name="rpool", bufs=2))
                opool = c2.enter_context(tc.tile_pool(name="opool", bufs=8))
                psS = c2.enter_context(tc.tile_pool(name="psS", bufs=2, space="PSUM"))
                psPV = c2.enter_context(tc.tile_pool(name="psPV", bufs=2, space="PSUM"))
                psO = c2.enter_context(tc.tile_pool(name="psO", bufs=2, space="PSUM"))

                evac_ctr = [0]

                def evac_o(o_ps, t0, n):
                    # alternate evacs over DVE/ACT by n-parity (gpsimd
                    # cannot read PSUM on TRN2). Even n (incl. the n=6
                    # filler right before a head's den tree finishes) goes
                    # to ACT so it cannot queue ahead of the tree's final
                    # adds on DVE; odd n (incl. n=7, emitted after the
                    # tree) goes to DVE.
                    o_t = opool.tile([128, 512], BF16)
                    if n % 2 == 0:
                        nc.scalar.activation(out=o_t, in_=o_ps, func=COPY)
                    else:
                        nc.vector.tensor_copy(o_t, o_ps)
                    nc.sync.dma_start(
                        out=out_r[t0:t0 + 128, n * 512:(n + 1) * 512],
                        in_=o_t)

                def outproj_nchunk(pb, pj, pa, tt2, n, o_ps=None):
                    # output projection for tq-tile tt2, H-chunk n, of chunk
                    # (pb, pj): accumulate the 4 heads in PSUM against woT.
                    t0 = pb * S + pj * 512 + tt2 * 128
                    if o_ps is None:
                        o_ps = psO.tile([128, 512], F32, name="o_ps")
                    for m in range(G):
                        nc.tensor.matmul(
                            o_ps, pa[m][:, tt2 * 128:(tt2 + 1) * 128],
                            wo_sb[:, m, n * 512:(n + 1) * 512],
                            start=(m == 0), stop=(m == G - 1))
                    evac_o(o_ps, t0, n)

                def flush_den(pend):
                    # softmax denominator of a finished head: one ones-matmul
                    # on the DVE tree sum, reciprocal, normalize into a_ch.
                    # The PSUM tile comes from the psO ring (outproj evacs
                    # release it fast) — a psS-ring tile would WAR-wait on
                    # the current head's exp(0), stalling the in-order PE.
                    acc512, pv_ps, a_t = pend
                    den_ps = psO.tile([128, 512], F32, name="o_ps")
                    nc.tensor.matmul(den_ps, ones_sb, acc512,
                                     start=True, stop=True)
                    rec_t = rpool.tile([128, 512], F32)
                    nc.vector.reciprocal_approx_fast(out=rec_t, in_=den_ps)
                    nc.vector.tensor_mul(a_t, pv_ps, rec_t)

                pending = None   # den work of the previous head
                prev = None      # a_ch of the previous (b, j)
                for b in range(B):
                    for j in range(SJ):
                        tqsl = slice(b * S + j * 512, b * S + (j + 1) * 512)
                        a_ch = [apool.tile([128, 512], BF16, name=f"a_ch{m}")
                                for m in range(G)]
                        for m in range(G):
                            pv_ps = psPV.tile([128, 512], F32,
                                              name="pv_ps")
                            p_list = []
                            t_parts = []
                            for g in range(NG):
                                s_ps = psS.tile([128, 1024], F32, name="s_ps")
                                for h in range(2):
                                    ti = b * SI + 2 * g + h
                                    nc.tensor.matmul(
                                        s_ps[:, h * 512:(h + 1) * 512],
                                        k_sb[:, ti * 128:(ti + 1) * 128],
                                        q_sb[:, m, tqsl], start=True, stop=True)
                                p_t = ppool.tile([128, 1024], BF16)
                                nc.scalar.activation(out=p_t, in_=s_ps,
                                                     func=EXP, scale=SCALE)
                                p_list.append(p_t)
                                if g % 2 == 1:
                                    tk = tpool.tile([128, 1024], BF16,
                                                    name=f"t{g // 2}")
                                    nc.vector.tensor_add(tk, p_list[g - 1],
                                                         p_list[g])
                                    t_parts.append(tk)
                                    # pre-reduce the den tree as tiles land
                                    # so only t3 -> a1024 -> acc512 remain
                                    # after exp(7): the den matmul of the
                                    # next head stalled ~0.6us on this chain
                                    if g == 3:
                                        pre = tpool.tile([128, 1024], BF16,
                                                         name="pre")
                                        nc.vector.tensor_add(
                                            pre, t_parts[0], t_parts[1])
                                    elif g == 5:
                                        pre2 = tpool.tile([128, 1024], BF16,
                                                          name="pre2")
                                        nc.vector.tensor_add(
                                            pre2, pre, t_parts[2])
                                # den flush + interleaved output projection
                                # go BEFORE the PV pair: the PE is in-order,
                                # so exp-independent work must sit ahead of
                                # the exp-dependent PV matmuls to cover the
                                # ACT latency
                                # filler schedule. m==0 keeps the baseline
                                # order (den flush at g==1) because its
                                # g==1 outproj would read the prev chunk's
                                # a_ch[3], which this very flush writes.
                                # m>=1 move the den flush to g==2 with an
                                # outproj ahead of it at g==1: the previous
                                # head's DVE tree then has ~2.5us of PE work
                                # ahead of the den matmul instead of ~0.9us
                                # (den-waits-on-tree was most of the phase-2
                                # gap time), and the outproj also covers the
                                # exp(0) latency before PV(0).
                                if m == 0:
                                    if g == 1 and pending is not None:
                                        flush_den(pending)
                                        pending = None
                                    if g >= 2 and prev is not None:
                                        outproj_nchunk(prev[0], prev[1],
                                                       prev[2], m, g - 2)
                                else:
                                    if g == 1 and prev is not None:
                                        outproj_nchunk(prev[0], prev[1],
                                                       prev[2], m, 0)
                                    if g == 2 and pending is not None:
                                        flush_den(pending)
                                        pending = None
                                    if g >= 2 and prev is not None:
                                        outproj_nchunk(prev[0], prev[1],
                                                       prev[2], m, g - 1)
                                if g >= 1:
                                    pg = p_list[g - 1]
                                    for h in range(2):
                                        ti = b * SI + 2 * (g - 1) + h
                                        nc.tensor.matmul(
                                            pv_ps, v_sb[:, ti, :],
                                            pg[:, h * 512:(h + 1) * 512],
                                            start=(g == 1 and h == 0),
                                            stop=False)
                            if m == 0 and prev is not None:
                                outproj_nchunk(prev[0], prev[1], prev[2],
                                               m, NJ - 2)
                            pg = p_list[NG - 1]
                            for h in range(2):
                                ti = b * SI + 2 * (NG - 1) + h
                                nc.tensor.matmul(
                                    pv_ps, v_sb[:, ti, :],
                                    pg[:, h * 512:(h + 1) * 512],
                                    start=False, stop=(h == 1))
                            # finish the denominator tree on DVE
                            a1024 = tpool.tile([128, 1024], BF16, name="a1024")
                            nc.vector.tensor_add(a1024, pre2, t_parts[3])
                            acc512 = tpool.tile([128, 512], BF16, name="a512")
                            nc.vector.tensor_add(acc512, a1024[:, 0:512],
                                                 a1024[:, 512:1024])
                            pending = (acc512, pv_ps, a_ch[m])
                            # post-pending filler for every head: covers the
                            # NEXT head's den matmul (m==0: tree of the prev
                            # chunk's last head) against the DVE tree latency
                            if prev is not None:
                                outproj_nchunk(prev[0], prev[1], prev[2],
                                               m, NJ - 1)
                        prev = (b, j, a_ch)
                # ---- tail: last chunk's den flush + output projection ----
                # Open two O groups with heads 0-2 first (their a_ch are
                # ready) so the in-order PE isn't stalled behind the last
                # head's den chain (DVE tree -> ones-matmul -> recip -> mul).
                # The den matmul uses a psPV-ring tile (PV is finished).
                pb, pj, pa = prev
                acc512, pv_ps, a_t = pending

                def tail_slot(i):
                    # 4-deep psum rotation for the tail: psO's 2 slots plus
                    # the (now idle) psS pool's 2 slots, so group i+4 waits
                    # on evac(i) with 3 groups of slack instead of 1
                    if i % 4 < 2:
                        return psO.tile([128, 512], F32, name="o_ps")
                    return psS.tile([128, 1024], F32, name="s_ps")[:, 0:512]

                den_ps = psPV.tile([128, 512], F32, name="pv_ps")
                nc.tensor.matmul(den_ps, ones_sb, acc512, start=True,
                                 stop=True)
                rec_t = rpool.tile([128, 512], F32)
                nc.vector.reciprocal_approx_fast(out=rec_t, in_=den_ps)
                nc.vector.tensor_mul(a_t, pv_ps, rec_t)
                ti = 0
                for tt2 in range(4):
                    for n in range(NJ):
                        outproj_nchunk(pb, pj, pa, tt2, n,
                                       o_ps=tail_slot(ti))
                        ti += 1
    nc.compile()
    return nc


_NC_CACHE = None


def _get_nc():
    global _NC_CACHE
    if _NC_CACHE is None:
        _NC_CACHE = build_nc()
    return _NC_CACHE


def make_in_maps(x, wq, wk, wv, wo):
    xt = np.ascontiguousarray(x.reshape(T, H).T).astype(bfloat16)
    wqb = wq.astype(bfloat16)
    wkb = wk.astype(bfloat16)
    wvb = wv.astype(bfloat16)
    wob = wo.astype(bfloat16)
    ones = np.ones((128, 128), dtype=bfloat16)
    in_maps = []
    for c in range(NCORES):
        qsl = slice(c * G * D, (c + 1) * G * D)
        ksl = slice(c * D, (c + 1) * D)
        in_maps.append({
            "xt": xt,
            "wqt": np.ascontiguousarray(wqb[qsl, :].T),
            "wkt": np.ascontiguousarray(wkb[ksl, :].T),
            "wvt": np.ascontiguousarray(wvb[ksl, :].T),
            "wot": np.ascontiguousarray(wob[:, qsl].T),
            "ones": ones,
        })
    return in_maps


def kernel(x, wq, wk, wv, wo, **run_kwargs):
    nc = _get_nc()
    in_maps = make_in_maps(np.asarray(x, dtype=np.float32),
                           np.asarray(wq, dtype=np.float32),
                           np.asarray(wk, dtype=np.float32),
                           np.asarray(wv, dtype=np.float32),
                           np.asarray(wo, dtype=np.float32))
    res = run_bass_kernel_spmd(nc, in_maps, core_ids=list(range(NCORES)),
                               **run_kwargs)
    acc = np.zeros((T, H), dtype=np.float32)
    for c in range(NCORES):
        acc += res.results[c]["out"].astype(np.float32)
    out = acc.reshape(B, S, H)
    if run_kwargs:
        return out, res
    return out

